# revision 39
# baseline (speedup 1.0000x reference)
"""Causal multi-head attention on 8 trn2 NeuronCores.

Sharding: core c handles batch b=c//4 and heads [4*(c%4), 4*(c%4)+4).
Each core computes its 4 heads' attention plus the partial output
projection against the matching 256 rows of Wo; the host sums the 4
partials per batch (the all-reduce implied by row-sharding Wo) and adds
bo.

v11 (on top of v5's bf16 matmuls / fused biases / denominator-column
softmax / lagged ctx pipeline). Measured ~172-177us vs the 204.8us v5
baseline (run-to-run spread is DVFS throttling, ~0.78-0.82 avg util
limit in the NTFF counters):
  - Input DMA rebuilt: weights packed host-side so each tensor lands in
    one large HW-DGE transfer (2-4KB per partition line). Each queue is
    latency-bound near ~130-150GB/s while all 8 cores load their inputs
    (chip-level HBM contention), so xt streams across all THREE queues
    in demand order: chunks 0-1 scalar-HW, 2-3 gpsimd-SW, 4-7 sync-HW
    behind the pair-0 weights; pair-1 weights and Wv/Wo trail.
  - Q/K projections run c-outer/sb-inner (8 PSUM banks: 4 Q + 4 K per
    pair) so the PE consumes xt chunk-by-chunk right behind the DMA.
  - The two heads of a pair run TOGETHER through attention, chunk by
    chunk: their scores share one 2-bank PSUM tile, one rectangular
    strided ACT instruction exponentiates both halves with zero wasted
    columns (ACTIVATE's ~340ns fixed cost made one-exp-per-chunk the
    attention-phase limiter), one DVE op masks both causal triangles
    (replacing v5's identity-matmul -1e9 adds: ~20k PE columns + their
    LDWEIGHTS), and the pair's softmax normalization shares a single
    broadcast + reciprocal. Masking precedes the ones-column ctx
    matmul, so the denominator stays exact.
  - Denominator broadcast split into two rank-1 sel-matmuls whose den
    rows are copied in parallel (DVE + ACT) so nothing serializes the
    pair-boundary norm chain; ctx trails its exp by four chunks.
  - Output-projection slots for the previous query block are spread
    evenly through the score/ctx stream (the PE fills the gaps the exp
    pipeline leaves), evictions alternate ACT/DVE, and stores alternate
    the sync/scalar DMA queues. Output is fp16 (halves the 8MB/core
    store; fp16 beats bf16 on precision at this scale), one [128,1024]
    DMA per row block.
"""

import sys

for _p in ("/opt/trn_rl_repo", "/root/.axon_site/_ro/trn_rl_repo"):
    if _p not in sys.path:
        sys.path.insert(0, _p)

import numpy as np

import concourse.bass as bass
import concourse.bacc as bacc
import concourse.tile as tile
from concourse import mybir
from concourse.bass_utils import run_bass_kernel_spmd

F32 = mybir.dt.float32
F16 = mybir.dt.float16
BF16 = mybir.dt.bfloat16

B, S, D, H, DK = 2, 2048, 1024, 16, 64
NCORES = 8
HPC = 4          # heads per core
NPAIR = 2        # head pairs per core
ND = D // 128    # 8 contraction chunks over d
NS = S // 512    # 4 query blocks
NS16 = S // 128  # 16 sequence chunks

_CACHE = {}


def _build_bass():
    nc = bacc.Bacc(None)
    xt = nc.dram_tensor("xt", [128, ND, S], BF16, kind="ExternalInput")
    wq = nc.dram_tensor("wq", [128, NPAIR, ND, 128], BF16, kind="ExternalInput")
    wk = nc.dram_tensor("wk", [128, NPAIR, ND, 128], BF16, kind="ExternalInput")
    wv = nc.dram_tensor("wv", [128, ND, 256], BF16, kind="ExternalInput")
    wo = nc.dram_tensor("wo", [128, 2, D], BF16, kind="ExternalInput")
    bq = nc.dram_tensor("bq", [128, NPAIR], F32, kind="ExternalInput")
    bv_bc = nc.dram_tensor("bv_bc", [128, NPAIR, 2, 64], F32, kind="ExternalInput")
    mask01 = nc.dram_tensor("mask01", [128, 128], BF16, kind="ExternalInput")
    sel = nc.dram_tensor("sel", [2, 128], BF16, kind="ExternalInput")
    vfix = nc.dram_tensor("vfix", [128, 64], BF16, kind="ExternalInput")
    out = nc.dram_tensor("out", [S, D], F16, kind="ExternalOutput")

    with nc.allow_low_precision("bf16 operands; accumulation stays fp32 in PSUM"), \
            tile.TileContext(nc) as tc:
        with (
            tc.tile_pool(name="consts", bufs=1) as consts,
            tc.tile_pool(name="qkv", bufs=1) as qkv,
        ):
            wq_sb = consts.tile([128, NPAIR, ND, 128], BF16, tag="wq")
            wk_sb = consts.tile([128, NPAIR, ND, 128], BF16, tag="wk")
            wv_sb = consts.tile([128, ND, 256], BF16, tag="wv")
            wo_sb = consts.tile([128, 2, D], BF16, tag="wo")
            bq_sb = consts.tile([128, NPAIR], F32, tag="bq")
            bv_sb = consts.tile([128, NPAIR, 2, 64], F32, tag="bv")
            mask_sb = consts.tile([128, 128], BF16, tag="mask01")
            sela_sb = consts.tile([1, 128], BF16, tag="sela")
            selb_sb = consts.tile([1, 128], BF16, tag="selb")

            qt_sb = qkv.tile([128, NPAIR, S], BF16, tag="qt")
            kt_sb = qkv.tile([128, NPAIR, S], BF16, tag="kt")
            # Vaug per pair: cols 0:64 V_even | 64 ones | 65:128 zeros
            # | 128:192 V_odd. Even lhsT = cols 0:65 -> ctx on parts
            # 0:64 (+denominator row 64); odd lhsT = cols 64:192 ->
            # denominator on part 0, ctx on parts 64:128.
            va_sb = qkv.tile([128, NPAIR, NS16, 192], BF16, tag="va")
            ctxcat_sb = qkv.tile([128, 2, S], BF16, tag="ctxcat")

            # xt lives in the outer pool: the V projection now runs
            # interleaved with the attention phase and reads it there
            xt_sb = qkv.tile([128, ND, S], BF16, tag="xt")
            with (
                tc.tile_pool(name="mmp", bufs=8, space="PSUM") as mmp,
            ):
                # xt per-chunk on the scalar HW-DGE queue, weights as one
                # large DMA each on the sync HW queue, tiny consts on the
                # gpsimd SW queue: three queues stream in parallel and
                # every HW transfer moves 2-4KB per partition line.
                nc.scalar.dma_start(out=xt_sb[:, 0, :], in_=xt[:, 0, :])
                nc.scalar.dma_start(out=xt_sb[:, 1, :], in_=xt[:, 1, :])
                nc.gpsimd.dma_start(out=xt_sb[:, 2, :], in_=xt[:, 2, :])
                nc.gpsimd.dma_start(out=xt_sb[:, 3, :], in_=xt[:, 3, :])
                nc.sync.dma_start(out=wq_sb[:, 0], in_=wq[:, 0])
                nc.sync.dma_start(out=wk_sb[:, 0], in_=wk[:, 0])
                nc.gpsimd.dma_start(out=bq_sb[:], in_=bq[:])
                for c in range(4, ND):
                    nc.sync.dma_start(out=xt_sb[:, c, :], in_=xt[:, c, :])
                nc.sync.dma_start(out=wq_sb[:, 1], in_=wq[:, 1])
                nc.sync.dma_start(out=wk_sb[:, 1], in_=wk[:, 1])
                nc.sync.dma_start(out=wv_sb[:], in_=wv[:])
                nc.sync.dma_start(out=wo_sb[:], in_=wo[:])
                nc.gpsimd.dma_start(out=bv_sb[:], in_=bv_bc[:])
                nc.gpsimd.dma_start(out=mask_sb[:], in_=mask01[:])
                nc.gpsimd.dma_start(out=sela_sb[:], in_=sel[0:1, :])
                nc.gpsimd.dma_start(out=selb_sb[:], in_=sel[1:2, :])
                for p in range(NPAIR):
                    vfix_bc = bass.AP(
                        tensor=vfix.ap().tensor,
                        offset=0,
                        ap=[[64, 128], [0, NS16], [1, 64]],
                    )
                    nc.gpsimd.dma_start(out=va_sb[:, p, :, 64:128], in_=vfix_bc)

                # ---- Q^T / K^T projections (per pair, dk on partitions).
                # c-outer so the PE wants xt chunk c only ~1.7us after
                # chunk c-1: it trails right behind the streaming DMA.
                for p in range(NPAIR):
                    qps = [
                        mmp.tile([128, 512], F32, tag="mm", name=f"qp{sb}")
                        for sb in range(NS)
                    ]
                    kps = [
                        mmp.tile([128, 512], F32, tag="mm", name=f"kp{sb}")
                        for sb in range(NS)
                    ]
                    for c in range(ND):
                        for sb in range(NS):
                            nc.tensor.matmul(
                                qps[sb][:],
                                lhsT=wq_sb[:, p, c, :],
                                rhs=xt_sb[:, c, sb * 512:(sb + 1) * 512],
                                start=(c == 0),
                                stop=(c == ND - 1),
                            )
                        for sb in range(NS):
                            nc.tensor.matmul(
                                kps[sb][:],
                                lhsT=wk_sb[:, p, c, :],
                                rhs=xt_sb[:, c, sb * 512:(sb + 1) * 512],
                                start=(c == 0),
                                stop=(c == ND - 1),
                            )
                    for sb in range(NS):
                        nc.scalar.activation(
                            out=qt_sb[:, p, sb * 512:(sb + 1) * 512],
                            in_=qps[sb][:],
                            func=mybir.ActivationFunctionType.Identity,
                            bias=bq_sb[:, p:p + 1],
                            scale=1.0,
                        )
                        nc.vector.tensor_copy(
                            out=kt_sb[:, p, sb * 512:(sb + 1) * 512],
                            in_=kps[sb][:],
                        )

            # ---- attention + output projection, per query block.
            # The two heads of a pair run TOGETHER chunk-by-chunk: their
            # scores share one 2-bank PSUM tile (even head bank 0, odd
            # bank 1), one rectangular strided ACT instruction
            # exponentiates both with zero wasted columns, one DVE op
            # masks both diagonal triangles, and the pair's softmax
            # normalization shares a single sel-matmul broadcast +
            # reciprocal. Output-projection slots for the previous qb are
            # spread evenly through the score/ctx stream so the PE fills
            # the gaps that the exp pipeline leaves.
            with (
                tc.tile_pool(name="stp", bufs=2, space="PSUM") as stp,
                tc.tile_pool(name="ctxp", bufs=2, space="PSUM") as ctxp,
                tc.tile_pool(name="vpp", bufs=1, space="PSUM") as vpp,
                tc.tile_pool(name="ptp", bufs=6) as ptp,
                tc.tile_pool(name="smp", bufs=3) as smp,
                tc.tile_pool(name="outp", bufs=3) as outp,
            ):
                def emit_vblock(s0):
                    # V projection super-block (two seq row-blocks in one
                    # PSUM bank), in natural [s, dk] layout, 4 heads at
                    # once. Interleaved into the attention stream: qb's
                    # ctx only needs va rows < (qb+1)*4, so block s0 for
                    # qb+1 rides inside qb's score/ctx stream, filling
                    # the PE gaps the exp pipeline leaves. bv is added
                    # during the eviction (tensor_add with a
                    # partition-broadcast constant): exact through the
                    # softmax denominator trick since rows of P sum to
                    # den.
                    vp = vpp.tile([128, 2, 256], F32, tag="vp", name="vp")
                    for bi in range(2):
                        s16 = s0 + bi
                        for c in range(ND):
                            nc.tensor.matmul(
                                vp[:, bi, :],
                                lhsT=xt_sb[:, c, s16 * 128:(s16 + 1) * 128],
                                rhs=wv_sb[:, c, :],
                                start=(c == 0),
                                stop=(c == ND - 1),
                            )
                        # V_even -> va cols 0:64, V_odd -> cols 128:192
                        # in one two-segment add per pair
                        for p2 in range(NPAIR):
                            d0 = va_sb[:, p2, s16, 0:64]
                            dst = bass.AP(
                                tensor=d0.tensor, offset=d0.offset,
                                ap=[[d0.ap[0][0], 128], [128, 2], [1, 64]],
                            )
                            s0v = vp[:, bi, p2 * 128:(p2 + 1) * 128]
                            srcv = bass.AP(
                                tensor=s0v.tensor, offset=s0v.offset,
                                ap=[[s0v.ap[0][0], 128], [64, 2], [1, 64]],
                            )
                            nc.vector.tensor_add(
                                out=dst, in0=srcv, in1=bv_sb[:, p2, :, :]
                            )
                def emit_norm_pair(ctx_e, ctx_o, den_e, den_o, p, qb):
                    # two rank-1 broadcast matmuls serve both heads: sel_a
                    # routes den_e to partitions 0:64, sel_b routes den_o
                    # to 64:128, matching the ctx parity layout. The two
                    # den copies live in separate tiles so DVE and ACT
                    # produce them in parallel. custom-DVE ops (and
                    # tile_position=(0,64) matmuls) misbehave on HW when
                    # based at partition 64, so everything stays at base 0.
                    bc_ps = ctxp.tile([128, 512], F32, tag="op", name="bc_ps", bufs=1)
                    nc.tensor.matmul(
                        bc_ps[:],
                        lhsT=sela_sb[:],
                        rhs=den_e[:],
                        start=True,
                        stop=False,
                    )
                    nc.tensor.matmul(
                        bc_ps[:],
                        lhsT=selb_sb[:],
                        rhs=den_o[:],
                        start=False,
                        stop=True,
                    )
                    rcp = smp.tile([128, 512], F32, tag="rcp", name="rcp")
                    nc.vector.reciprocal_approx_fast(out=rcp[:], in_=bc_ps[:])
                    nc.vector.tensor_mul(
                        out=ctxcat_sb[0:64, p, qb * 512:(qb + 1) * 512],
                        in0=ctx_e[0:64, :],
                        in1=rcp[0:64, :],
                    )
                    nc.vector.tensor_mul(
                        out=ctxcat_sb[64:128, p, qb * 512:(qb + 1) * 512],
                        in0=ctx_o[64:128, :],
                        in1=rcp[64:128, :],
                    )

                ot_tiles = {}

                def emit_op_slot(s16, do):
                    # one (row-block, output-half) slice of the output
                    # projection: two accumulating matmuls, an eviction,
                    # and (on the second half) the store
                    if do == 0:
                        ot_tiles[s16] = outp.tile(
                            [128, D], F16, tag="ot", name="ot"
                        )
                    ot = ot_tiles[s16]
                    op = ctxp.tile([128, 512], F32, tag="op", name="op", bufs=1)
                    nc.tensor.matmul(
                        op[:],
                        lhsT=ctxcat_sb[:, 0, s16 * 128:(s16 + 1) * 128],
                        rhs=wo_sb[:, 0, do * 512:(do + 1) * 512],
                        start=True,
                        stop=False,
                    )
                    nc.tensor.matmul(
                        op[:],
                        lhsT=ctxcat_sb[:, 1, s16 * 128:(s16 + 1) * 128],
                        rhs=wo_sb[:, 1, do * 512:(do + 1) * 512],
                        start=False,
                        stop=True,
                    )
                    if do == 0:
                        nc.scalar.copy(out=ot[:, 0:512], in_=op[:])
                    else:
                        nc.vector.tensor_copy(out=ot[:, 512:1024], in_=op[:])
                        oq = nc.sync if s16 % 2 == 0 else nc.scalar
                        oq.dma_start(
                            out=out[s16 * 128:(s16 + 1) * 128, :], in_=ot[:]
                        )

                emit_vblock(0)
                pending = None  # (ctx_e, ctx_o, den_e, den_o, p, qb)
                slots = []  # outproj work carried across query blocks
                for qb in range(NS):
                    nch = (qb + 1) * 4
                    if qb > 0:
                        slots += [(s16, do)
                                  for s16 in range((qb - 1) * 4, qb * 4)
                                  for do in range(2)]
                    # later query blocks have longer ACT-gated score
                    # sweeps and no V filler, so save outproj slots for
                    # them: drain ~6 per middle qb, everything in the last
                    want = len(slots) if qb == NS - 1 else min(len(slots), 6)
                    stride = max(1, (2 * nch) // want) if want else 1
                    drained = 0
                    for p in range(NPAIR):
                        qs_e = qt_sb[0:64, p, qb * 512:(qb + 1) * 512]
                        qs_o = qt_sb[64:128, p, qb * 512:(qb + 1) * 512]
                        ctx_e = ctxp.tile([128, 512], F32, tag="ctx", name="ctx_e")
                        ctx_o = ctxp.tile([128, 512], F32, tag="ctx", name="ctx_o")
                        lagged = []  # (pt, f0, diag, c) awaiting ctx mms

                        def emit_ctx(lag, ctx_e=ctx_e, ctx_o=ctx_o, p=p, nch=nch):
                            pt, f0, diag, c = lag
                            oe = ctx_e[0:65, f0:512] if diag else ctx_e[0:65, :]
                            nc.tensor.matmul(
                                oe,
                                lhsT=va_sb[:, p, c, 0:65],
                                rhs=pt[:, f0:512],
                                start=(c == 0),
                                stop=(c == nch - 1),
                            )
                            oo = ctx_o[:, f0:512] if diag else ctx_o[:]
                            nc.tensor.matmul(
                                oo,
                                lhsT=va_sb[:, p, c, 64:192],
                                rhs=pt[:, 512 + f0:1024],
                                start=(c == 0),
                                stop=(c == nch - 1),
                            )

                        for c in range(nch):
                            st = stp.tile([128, 1024], F32, tag="st", name="st")
                            pt = ptp.tile([128, 1024], BF16, tag="pt", name="pt")
                            diag = c >= qb * 4
                            # columns [0, f0) of a diag block are fully
                            # masked (q < kv everywhere): skip them.
                            f0 = 128 * (c - qb * 4) if diag else 0
                            nc.tensor.matmul(
                                st[:, f0:512],
                                lhsT=kt_sb[0:64, p, c * 128:(c + 1) * 128],
                                rhs=qs_e[:, f0:512],
                                start=True,
                                stop=True,
                            )
                            nc.tensor.matmul(
                                st[:, 512 + f0:1024],
                                lhsT=kt_sb[64:128, p, c * 128:(c + 1) * 128],
                                rhs=qs_o[:, f0:512],
                                start=True,
                                stop=True,
                            )
                            # one exp over both heads' halves: rectangular
                            # strided AP, no garbage columns
                            sv = st[:, f0:512]
                            pv = pt[:, f0:512]
                            nc.scalar.activation(
                                out=bass.AP(
                                    tensor=pv.tensor, offset=pv.offset,
                                    ap=[[pv.ap[0][0], 128], [512, 2],
                                        [1, 512 - f0]],
                                ),
                                in_=bass.AP(
                                    tensor=sv.tensor, offset=sv.offset,
                                    ap=[[sv.ap[0][0], 128], [512, 2],
                                        [1, 512 - f0]],
                                ),
                                func=mybir.ActivationFunctionType.Exp,
                            )
                            if diag:
                                # zero both heads' masked triangles in one
                                # DVE op; masking precedes the ones-column
                                # ctx matmul so the denominator stays exact
                                blk = pt[:, f0:f0 + 128]
                                mdst = bass.AP(
                                    tensor=blk.tensor, offset=blk.offset,
                                    ap=[[blk.ap[0][0], 128], [512, 2],
                                        [1, 128]],
                                )
                                msk = bass.AP(
                                    tensor=mask_sb.tensor,
                                    offset=mask_sb.offset,
                                    ap=[[mask_sb.ap[0][0], 128], [0, 2],
                                        [1, 128]],
                                )
                                nc.vector.tensor_mul(
                                    out=mdst, in0=mdst, in1=msk
                                )
                            # ctx trails four chunks behind its exp so the
                            # PE queue stays deep (hides ACT latency,
                            # semaphore propagation, and LDWEIGHTS)
                            lagged.append((pt, f0, diag, c))
                            while len(lagged) > 4:
                                emit_ctx(lagged.pop(0))
                            nf = 2 if nch <= 4 else 4
                            if c == nf and pending is not None:
                                emit_norm_pair(*pending)
                                pending = None
                            g = p * nch + c
                            if (drained < want and g >= 5
                                    and (g - 5) % stride == 0):
                                emit_op_slot(*slots.pop(0))
                                drained += 1
                            if qb == 0 and p == 0 and c == 1:
                                # second half of qb0's V rides inside its
                                # own score stream (fills the wait for the
                                # first super-block's DVE evictions)
                                emit_vblock(2)
                            if c == nch // 2 and qb < NS - 1:
                                # one V super-block for qb+1 per pair
                                emit_vblock((qb + 1) * 4 + p * 2)
                        for lag in lagged:
                            emit_ctx(lag)

                        # denominator rows -> two independent [1,512]
                        # tiles so the copies run in parallel (even on
                        # DVE, odd on ACT) and nothing serializes the
                        # broadcast matmuls at the next pair boundary
                        den_e = smp.tile([1, 512], BF16, tag="den", name="den_e")
                        den_o = smp.tile([1, 512], BF16, tag="deno", name="den_o")
                        nc.vector.tensor_copy(
                            out=den_e[:], in_=ctx_e[64:65, :]
                        )
                        nc.scalar.copy(
                            out=den_o[:], in_=ctx_o[0:1, :]
                        )
                        pending = (ctx_e, ctx_o, den_e, den_o, p, qb)

                emit_norm_pair(*pending)
                pending = None
                for s16, do in slots:
                    emit_op_slot(s16, do)
                for s16 in range((NS - 1) * 4, NS * 4):
                    emit_op_slot(s16, 0)
                    emit_op_slot(s16, 1)
    if not nc.is_finalized():
        nc.finalize()
    return nc


def _prep_inputs(embeddings, Wq, bq, Wk, bk, Wv, bv, Wo, bo):
    embeddings = np.asarray(embeddings, np.float32)
    Wq, bq = np.asarray(Wq, np.float32), np.asarray(bq, np.float32)
    Wk = np.asarray(Wk, np.float32)
    Wv, bv = np.asarray(Wv, np.float32), np.asarray(bv, np.float32)
    Wo = np.asarray(Wo, np.float32)

    import ml_dtypes
    bf16_t = ml_dtypes.bfloat16
    # mask01[p, j] = 1 where col j (query) >= partition p (key) in the
    # 128x128 diagonal block, else 0
    mask01 = np.triu(np.ones((128, 128), np.float32)).astype(bf16_t)
    vfix = np.zeros((128, 64), np.float32)
    # two ones columns: va col 64 puts the denominator on partition 64 for
    # even heads / partition 0 for odd heads; va col 65 duplicates it on
    # partition 1 so the odd-head den copy needs no partition shift
    vfix[:, 0] = 1.0
    vfix[:, 1] = 1.0
    vfix = vfix.astype(bf16_t)
    sel = np.zeros((2, 128), np.float32)
    sel[0, 0:64] = 1.0
    sel[1, 64:128] = 1.0
    sel = sel.astype(bf16_t)

    in_maps = []
    for c in range(NCORES):
        b, g = c // 4, c % 4
        hs = HPC * g
        # [128, ND, S]: partition = d % 128, chunk = d // 128
        xt = np.ascontiguousarray(
            embeddings[b].T.reshape(ND, 128, S).transpose(1, 0, 2)
        ).astype(bf16_t)
        # 1/sqrt(dk) folded into Wq/bq (exact power of two)
        wq2 = np.stack(
            [np.concatenate([Wq[hs + 2 * p], Wq[hs + 2 * p + 1]], axis=1)
             for p in range(NPAIR)]
        ) * 0.125
        wk2 = np.stack(
            [np.concatenate([Wk[hs + 2 * p], Wk[hs + 2 * p + 1]], axis=1)
             for p in range(NPAIR)]
        )
        # [NPAIR, D, 128] -> [128, NPAIR, ND, 128] single-DMA layout
        wq2 = wq2.reshape(NPAIR, ND, 128, 128).transpose(2, 0, 1, 3)
        wk2 = wk2.reshape(NPAIR, ND, 128, 128).transpose(2, 0, 1, 3)
        wv4 = np.concatenate([Wv[hs + h] for h in range(HPC)], axis=1)
        wv4 = wv4.reshape(ND, 128, 256).transpose(1, 0, 2)
        wo4 = Wo[hs * DK:(hs + HPC) * DK, :].reshape(2, 128, D).transpose(1, 0, 2)
        bq2 = np.stack(
            [np.concatenate([bq[hs + 2 * p], bq[hs + 2 * p + 1]]) / 8.0
             for p in range(NPAIR)], axis=1
        )
        bvb = np.zeros((128, NPAIR, 2, 64), np.float32)
        for p in range(NPAIR):
            bvb[:, p, 0, :] = bv[hs + 2 * p][None, :]
            bvb[:, p, 1, :] = bv[hs + 2 * p + 1][None, :]
        in_maps.append({
            "xt": xt,
            "wq": np.ascontiguousarray(wq2).astype(bf16_t),
            "wk": np.ascontiguousarray(wk2).astype(bf16_t),
            "wv": np.ascontiguousarray(wv4).astype(bf16_t),
            "wo": np.ascontiguousarray(wo4).astype(bf16_t),
            "bq": np.ascontiguousarray(bq2),
            "bv_bc": bvb,
            "mask01": mask01,
            "sel": sel,
            "vfix": vfix,
        })
    return in_maps


def kernel(embeddings, Wq, bq, Wk, bk, Wv, bv, Wo, bo, _trace=False, _trace_kw=None):
    if "nc" not in _CACHE:
        _CACHE["nc"] = _build_bass()
    nc = _CACHE["nc"]
    in_maps = _prep_inputs(embeddings, Wq, bq, Wk, bk, Wv, bv, Wo, bo)
    kw = dict(_trace_kw or {})
    res = run_bass_kernel_spmd(
        nc, in_maps, core_ids=list(range(NCORES)), trace=_trace, **kw
    )
    _CACHE["last_result"] = res
    bo32 = np.asarray(bo, np.float32)
    out = np.empty((B, S, D), np.float32)
    for b in range(B):
        acc = np.asarray(res.results[4 * b]["out"], np.float32).copy()
        for g in range(1, 4):
            acc += np.asarray(res.results[4 * b + g]["out"], np.float32)
        out[b] = acc + bo32
    return out


# revision 40
# speedup vs baseline: 1.0573x; 1.0573x over previous
"""Causal multi-head attention on 8 trn2 NeuronCores.

Sharding: core c handles batch b=c//4 and heads [4*(c%4), 4*(c%4)+4).
Each core computes its 4 heads' attention plus the partial output
projection against the matching 256 rows of Wo; the host sums the 4
partials per batch (the all-reduce implied by row-sharding Wo) and adds
bo.

v11 (on top of v5's bf16 matmuls / fused biases / denominator-column
softmax / lagged ctx pipeline). Measured ~172-177us vs the 204.8us v5
baseline (run-to-run spread is DVFS throttling, ~0.78-0.82 avg util
limit in the NTFF counters):
  - Input DMA rebuilt: weights packed host-side so each tensor lands in
    one large HW-DGE transfer (2-4KB per partition line). Each queue is
    latency-bound near ~130-150GB/s while all 8 cores load their inputs
    (chip-level HBM contention), so xt streams across all THREE queues
    in demand order: chunks 0-1 scalar-HW, 2-3 gpsimd-SW, 4-7 sync-HW
    behind the pair-0 weights; pair-1 weights and Wv/Wo trail.
  - Q/K projections run c-outer/sb-inner (8 PSUM banks: 4 Q + 4 K per
    pair) so the PE consumes xt chunk-by-chunk right behind the DMA.
  - The two heads of a pair run TOGETHER through attention, chunk by
    chunk: their scores share one 2-bank PSUM tile, one rectangular
    strided ACT instruction exponentiates both halves with zero wasted
    columns (ACTIVATE's ~340ns fixed cost made one-exp-per-chunk the
    attention-phase limiter), one DVE op masks both causal triangles
    (replacing v5's identity-matmul -1e9 adds: ~20k PE columns + their
    LDWEIGHTS), and the pair's softmax normalization shares a single
    broadcast + reciprocal. Masking precedes the ones-column ctx
    matmul, so the denominator stays exact.
  - Denominator broadcast split into two rank-1 sel-matmuls whose den
    rows are copied in parallel (DVE + ACT) so nothing serializes the
    pair-boundary norm chain; ctx trails its exp by four chunks.
  - Output-projection slots for the previous query block are spread
    evenly through the score/ctx stream (the PE fills the gaps the exp
    pipeline leaves), evictions alternate ACT/DVE, and stores alternate
    the sync/scalar DMA queues. Output is fp16 (halves the 8MB/core
    store; fp16 beats bf16 on precision at this scale), one [128,1024]
    DMA per row block.
"""

import sys

for _p in ("/opt/trn_rl_repo", "/root/.axon_site/_ro/trn_rl_repo"):
    if _p not in sys.path:
        sys.path.insert(0, _p)

import numpy as np

import concourse.bass as bass
import concourse.bacc as bacc
import concourse.tile as tile
from concourse import mybir
from concourse.bass_utils import run_bass_kernel_spmd

F32 = mybir.dt.float32
F16 = mybir.dt.float16
BF16 = mybir.dt.bfloat16

B, S, D, H, DK = 2, 2048, 1024, 16, 64
NCORES = 8
HPC = 4          # heads per core
NPAIR = 2        # head pairs per core
ND = D // 128    # 8 contraction chunks over d
NS = S // 512    # 4 query blocks
NS16 = S // 128  # 16 sequence chunks

_CACHE = {}


def _build_bass():
    nc = bacc.Bacc(None)
    xt = nc.dram_tensor("xt", [128, ND, S], BF16, kind="ExternalInput")
    wq = nc.dram_tensor("wq", [128, NPAIR, ND, 128], BF16, kind="ExternalInput")
    wk = nc.dram_tensor("wk", [128, NPAIR, ND, 128], BF16, kind="ExternalInput")
    wv = nc.dram_tensor("wv", [128, ND, 256], BF16, kind="ExternalInput")
    wo = nc.dram_tensor("wo", [128, 2, D], BF16, kind="ExternalInput")
    bq = nc.dram_tensor("bq", [128, NPAIR], F32, kind="ExternalInput")
    bv_bc = nc.dram_tensor("bv_bc", [128, NPAIR, 2, 64], F32, kind="ExternalInput")
    mask01 = nc.dram_tensor("mask01", [128, 128], BF16, kind="ExternalInput")
    sel = nc.dram_tensor("sel", [2, 128], BF16, kind="ExternalInput")
    vfix = nc.dram_tensor("vfix", [128, 64], BF16, kind="ExternalInput")
    out = nc.dram_tensor("out", [S, D], F16, kind="ExternalOutput")

    with nc.allow_low_precision("bf16 operands; accumulation stays fp32 in PSUM"), \
            tile.TileContext(nc) as tc:
        with (
            tc.tile_pool(name="consts", bufs=1) as consts,
            tc.tile_pool(name="qkv", bufs=1) as qkv,
        ):
            wq_sb = consts.tile([128, NPAIR, ND, 128], BF16, tag="wq")
            wk_sb = consts.tile([128, NPAIR, ND, 128], BF16, tag="wk")
            wv_sb = consts.tile([128, ND, 256], BF16, tag="wv")
            wo_sb = consts.tile([128, 2, D], BF16, tag="wo")
            bq_sb = consts.tile([128, NPAIR], F32, tag="bq")
            bv_sb = consts.tile([128, NPAIR, 2, 64], F32, tag="bv")
            mask_sb = consts.tile([128, 128], BF16, tag="mask01")
            sela_sb = consts.tile([1, 128], BF16, tag="sela")
            selb_sb = consts.tile([1, 128], BF16, tag="selb")

            qt_sb = qkv.tile([128, NPAIR, S], BF16, tag="qt")
            kt_sb = qkv.tile([128, NPAIR, S], BF16, tag="kt")
            # Vaug per pair: cols 0:64 V_even | 64 ones | 65:128 zeros
            # | 128:192 V_odd. Even lhsT = cols 0:65 -> ctx on parts
            # 0:64 (+denominator row 64); odd lhsT = cols 64:192 ->
            # denominator on part 0, ctx on parts 64:128.
            va_sb = qkv.tile([128, NPAIR, NS16, 192], BF16, tag="va")
            ctxcat_sb = qkv.tile([128, 2, S], BF16, tag="ctxcat")

            # xt lives in the outer pool: the V projection now runs
            # interleaved with the attention phase and reads it there
            xt_sb = qkv.tile([128, ND, S], BF16, tag="xt")
            with (
                tc.tile_pool(name="mmp", bufs=8, space="PSUM") as mmp,
            ):
                # xt per-chunk on the scalar HW-DGE queue, weights as one
                # large DMA each on the sync HW queue, tiny consts on the
                # gpsimd SW queue: three queues stream in parallel and
                # every HW transfer moves 2-4KB per partition line.
                nc.scalar.dma_start(out=xt_sb[:, 0, :], in_=xt[:, 0, :])
                nc.scalar.dma_start(out=xt_sb[:, 1, :], in_=xt[:, 1, :])
                nc.gpsimd.dma_start(out=xt_sb[:, 2, :], in_=xt[:, 2, :])
                nc.gpsimd.dma_start(out=xt_sb[:, 3, :], in_=xt[:, 3, :])
                nc.sync.dma_start(out=wq_sb[:, 0], in_=wq[:, 0])
                nc.sync.dma_start(out=wk_sb[:, 0], in_=wk[:, 0])
                nc.gpsimd.dma_start(out=bq_sb[:], in_=bq[:])
                for c in range(4, ND):
                    nc.sync.dma_start(out=xt_sb[:, c, :], in_=xt[:, c, :])
                nc.sync.dma_start(out=wq_sb[:, 1], in_=wq[:, 1])
                nc.sync.dma_start(out=wk_sb[:, 1], in_=wk[:, 1])
                nc.sync.dma_start(out=wv_sb[:], in_=wv[:])
                nc.sync.dma_start(out=wo_sb[:], in_=wo[:])
                nc.gpsimd.dma_start(out=bv_sb[:], in_=bv_bc[:])
                nc.gpsimd.dma_start(out=mask_sb[:], in_=mask01[:])
                nc.gpsimd.dma_start(out=sela_sb[:], in_=sel[0:1, :])
                nc.gpsimd.dma_start(out=selb_sb[:], in_=sel[1:2, :])
                for p in range(NPAIR):
                    vfix_bc = bass.AP(
                        tensor=vfix.ap().tensor,
                        offset=0,
                        ap=[[64, 128], [0, NS16], [1, 64]],
                    )
                    nc.gpsimd.dma_start(out=va_sb[:, p, :, 64:128], in_=vfix_bc)

                # ---- Q^T / K^T projections (per pair, dk on partitions).
                # c-outer so the PE wants xt chunk c only ~1.7us after
                # chunk c-1: it trails right behind the streaming DMA.
                for p in range(NPAIR):
                    qps = [
                        mmp.tile([128, 512], F32, tag="mm", name=f"qp{sb}")
                        for sb in range(NS)
                    ]
                    kps = [
                        mmp.tile([128, 512], F32, tag="mm", name=f"kp{sb}")
                        for sb in range(NS)
                    ]
                    for c in range(ND):
                        for sb in range(NS):
                            nc.tensor.matmul(
                                qps[sb][:],
                                lhsT=wq_sb[:, p, c, :],
                                rhs=xt_sb[:, c, sb * 512:(sb + 1) * 512],
                                start=(c == 0),
                                stop=(c == ND - 1),
                            )
                        for sb in range(NS):
                            nc.tensor.matmul(
                                kps[sb][:],
                                lhsT=wk_sb[:, p, c, :],
                                rhs=xt_sb[:, c, sb * 512:(sb + 1) * 512],
                                start=(c == 0),
                                stop=(c == ND - 1),
                            )
                    for sb in range(NS):
                        nc.scalar.activation(
                            out=qt_sb[:, p, sb * 512:(sb + 1) * 512],
                            in_=qps[sb][:],
                            func=mybir.ActivationFunctionType.Identity,
                            bias=bq_sb[:, p:p + 1],
                            scale=1.0,
                        )
                        nc.vector.tensor_copy(
                            out=kt_sb[:, p, sb * 512:(sb + 1) * 512],
                            in_=kps[sb][:],
                        )

            # ---- attention + output projection, per query block.
            # The two heads of a pair run TOGETHER chunk-by-chunk: their
            # scores share one 2-bank PSUM tile (even head bank 0, odd
            # bank 1), one rectangular strided ACT instruction
            # exponentiates both with zero wasted columns, one DVE op
            # masks both diagonal triangles, and the pair's softmax
            # normalization shares a single sel-matmul broadcast +
            # reciprocal. Output-projection slots for the previous qb are
            # spread evenly through the score/ctx stream so the PE fills
            # the gaps that the exp pipeline leaves.
            with (
                tc.tile_pool(name="stp", bufs=2, space="PSUM") as stp,
                tc.tile_pool(name="ctxp", bufs=2, space="PSUM") as ctxp,
                tc.tile_pool(name="vpp", bufs=1, space="PSUM") as vpp,
                tc.tile_pool(name="ptp", bufs=6) as ptp,
                tc.tile_pool(name="smp", bufs=3) as smp,
                tc.tile_pool(name="outp", bufs=3) as outp,
            ):
                def emit_vblock(s0):
                    # V projection super-block (two seq row-blocks in one
                    # PSUM bank), in natural [s, dk] layout, 4 heads at
                    # once. Interleaved into the attention stream: qb's
                    # ctx only needs va rows < (qb+1)*4, so block s0 for
                    # qb+1 rides inside qb's score/ctx stream, filling
                    # the PE gaps the exp pipeline leaves. bv is added
                    # during the eviction (tensor_add with a
                    # partition-broadcast constant): exact through the
                    # softmax denominator trick since rows of P sum to
                    # den.
                    vp = vpp.tile([128, 2, 256], F32, tag="vp", name="vp")
                    for bi in range(2):
                        s16 = s0 + bi
                        for c in range(ND):
                            nc.tensor.matmul(
                                vp[:, bi, :],
                                lhsT=xt_sb[:, c, s16 * 128:(s16 + 1) * 128],
                                rhs=wv_sb[:, c, :],
                                start=(c == 0),
                                stop=(c == ND - 1),
                            )
                        # V_even -> va cols 0:64, V_odd -> cols 128:192
                        # in one two-segment add per pair
                        for p2 in range(NPAIR):
                            d0 = va_sb[:, p2, s16, 0:64]
                            dst = bass.AP(
                                tensor=d0.tensor, offset=d0.offset,
                                ap=[[d0.ap[0][0], 128], [128, 2], [1, 64]],
                            )
                            s0v = vp[:, bi, p2 * 128:(p2 + 1) * 128]
                            srcv = bass.AP(
                                tensor=s0v.tensor, offset=s0v.offset,
                                ap=[[s0v.ap[0][0], 128], [64, 2], [1, 64]],
                            )
                            nc.vector.tensor_add(
                                out=dst, in0=srcv, in1=bv_sb[:, p2, :, :]
                            )
                def emit_norm_pair(ctx_e, ctx_o, den_e, den_o, p, qb):
                    # two rank-1 broadcast matmuls serve both heads: sel_a
                    # routes den_e to partitions 0:64, sel_b routes den_o
                    # to 64:128, matching the ctx parity layout. The two
                    # den copies live in separate tiles so DVE and ACT
                    # produce them in parallel. custom-DVE ops (and
                    # tile_position=(0,64) matmuls) misbehave on HW when
                    # based at partition 64, so everything stays at base 0.
                    bc_ps = ctxp.tile([128, 512], F32, tag="op", name="bc_ps", bufs=1)
                    nc.tensor.matmul(
                        bc_ps[:],
                        lhsT=sela_sb[:],
                        rhs=den_e[:],
                        start=True,
                        stop=False,
                    )
                    nc.tensor.matmul(
                        bc_ps[:],
                        lhsT=selb_sb[:],
                        rhs=den_o[:],
                        start=False,
                        stop=True,
                    )
                    rcp = smp.tile([128, 512], F32, tag="rcp", name="rcp")
                    nc.vector.reciprocal_approx_fast(out=rcp[:], in_=bc_ps[:])
                    nc.vector.tensor_mul(
                        out=ctxcat_sb[0:64, p, qb * 512:(qb + 1) * 512],
                        in0=ctx_e[0:64, :],
                        in1=rcp[0:64, :],
                    )
                    nc.vector.tensor_mul(
                        out=ctxcat_sb[64:128, p, qb * 512:(qb + 1) * 512],
                        in0=ctx_o[64:128, :],
                        in1=rcp[64:128, :],
                    )

                ot_tiles = {}

                def emit_op_slot(s16, do):
                    # one (row-block, output-half) slice of the output
                    # projection: two accumulating matmuls, an eviction,
                    # and (on the second half) the store
                    if do == 0:
                        ot_tiles[s16] = outp.tile(
                            [128, D], F16, tag="ot", name="ot"
                        )
                    ot = ot_tiles[s16]
                    op = ctxp.tile([128, 512], F32, tag="op", name="op", bufs=1)
                    nc.tensor.matmul(
                        op[:],
                        lhsT=ctxcat_sb[:, 0, s16 * 128:(s16 + 1) * 128],
                        rhs=wo_sb[:, 0, do * 512:(do + 1) * 512],
                        start=True,
                        stop=False,
                    )
                    nc.tensor.matmul(
                        op[:],
                        lhsT=ctxcat_sb[:, 1, s16 * 128:(s16 + 1) * 128],
                        rhs=wo_sb[:, 1, do * 512:(do + 1) * 512],
                        start=False,
                        stop=True,
                    )
                    if do == 0:
                        # both halves evict on DVE: ACT is the pace-setter
                        # of the long score sweeps these slots ride in
                        nc.vector.tensor_copy(out=ot[:, 0:512], in_=op[:])
                    else:
                        nc.vector.tensor_copy(out=ot[:, 512:1024], in_=op[:])
                        oq = nc.sync if s16 % 2 == 0 else nc.scalar
                        oq.dma_start(
                            out=out[s16 * 128:(s16 + 1) * 128, :], in_=ot[:]
                        )

                emit_vblock(0)
                pending = None  # (ctx_e, ctx_o, den_e, den_o, p, qb)
                slots = []  # outproj work carried across query blocks
                for qb in range(NS):
                    nch = (qb + 1) * 4
                    if qb > 0:
                        slots += [(s16, do)
                                  for s16 in range((qb - 1) * 4, qb * 4)
                                  for do in range(2)]
                    # later query blocks have longer ACT-gated score
                    # sweeps and no V filler, so save outproj slots for
                    # them: drain ~6 per middle qb, everything in the last
                    want = len(slots) if qb == NS - 1 else min(len(slots), 6)
                    stride = max(1, (2 * nch) // want) if want else 1
                    drained = 0
                    for p in range(NPAIR):
                        qs_e = qt_sb[0:64, p, qb * 512:(qb + 1) * 512]
                        qs_o = qt_sb[64:128, p, qb * 512:(qb + 1) * 512]
                        ctx_e = ctxp.tile([128, 512], F32, tag="ctx", name="ctx_e")
                        ctx_o = ctxp.tile([128, 512], F32, tag="ctx", name="ctx_o")
                        lagged = []  # (pt, f0, diag, c) awaiting ctx mms

                        def emit_ctx(lag, ctx_e=ctx_e, ctx_o=ctx_o, p=p, nch=nch):
                            pt, f0, diag, c = lag
                            oe = ctx_e[0:65, f0:512] if diag else ctx_e[0:65, :]
                            nc.tensor.matmul(
                                oe,
                                lhsT=va_sb[:, p, c, 0:65],
                                rhs=pt[:, f0:512],
                                start=(c == 0),
                                stop=(c == nch - 1),
                            )
                            oo = ctx_o[:, f0:512] if diag else ctx_o[:]
                            nc.tensor.matmul(
                                oo,
                                lhsT=va_sb[:, p, c, 64:192],
                                rhs=pt[:, 512 + f0:1024],
                                start=(c == 0),
                                stop=(c == nch - 1),
                            )

                        for c in range(nch):
                            st = stp.tile([128, 1024], F32, tag="st", name="st")
                            pt = ptp.tile([128, 1024], BF16, tag="pt", name="pt")
                            diag = c >= qb * 4
                            # columns [0, f0) of a diag block are fully
                            # masked (q < kv everywhere): skip them.
                            f0 = 128 * (c - qb * 4) if diag else 0
                            nc.tensor.matmul(
                                st[:, f0:512],
                                lhsT=kt_sb[0:64, p, c * 128:(c + 1) * 128],
                                rhs=qs_e[:, f0:512],
                                start=True,
                                stop=True,
                            )
                            nc.tensor.matmul(
                                st[:, 512 + f0:1024],
                                lhsT=kt_sb[64:128, p, c * 128:(c + 1) * 128],
                                rhs=qs_o[:, f0:512],
                                start=True,
                                stop=True,
                            )
                            # one exp over both heads' halves: rectangular
                            # strided AP, no garbage columns
                            sv = st[:, f0:512]
                            pv = pt[:, f0:512]
                            nc.scalar.activation(
                                out=bass.AP(
                                    tensor=pv.tensor, offset=pv.offset,
                                    ap=[[pv.ap[0][0], 128], [512, 2],
                                        [1, 512 - f0]],
                                ),
                                in_=bass.AP(
                                    tensor=sv.tensor, offset=sv.offset,
                                    ap=[[sv.ap[0][0], 128], [512, 2],
                                        [1, 512 - f0]],
                                ),
                                func=mybir.ActivationFunctionType.Exp,
                            )
                            if diag:
                                # zero both heads' masked triangles in one
                                # DVE op; masking precedes the ones-column
                                # ctx matmul so the denominator stays exact
                                blk = pt[:, f0:f0 + 128]
                                mdst = bass.AP(
                                    tensor=blk.tensor, offset=blk.offset,
                                    ap=[[blk.ap[0][0], 128], [512, 2],
                                        [1, 128]],
                                )
                                msk = bass.AP(
                                    tensor=mask_sb.tensor,
                                    offset=mask_sb.offset,
                                    ap=[[mask_sb.ap[0][0], 128], [0, 2],
                                        [1, 128]],
                                )
                                nc.vector.tensor_mul(
                                    out=mdst, in0=mdst, in1=msk
                                )
                            # ctx trails four chunks behind its exp so the
                            # PE queue stays deep (hides ACT latency,
                            # semaphore propagation, and LDWEIGHTS)
                            lagged.append((pt, f0, diag, c))
                            while len(lagged) > 4:
                                emit_ctx(lagged.pop(0))
                            nf = 2 if nch <= 4 else 4
                            if c == nf and pending is not None:
                                emit_norm_pair(*pending)
                                pending = None
                            g = p * nch + c
                            if (drained < want and g >= 5
                                    and (g - 5) % stride == 0):
                                emit_op_slot(*slots.pop(0))
                                drained += 1
                            if qb == 0 and p == 0 and c == 1:
                                # second half of qb0's V rides inside its
                                # own score stream (fills the wait for the
                                # first super-block's DVE evictions)
                                emit_vblock(2)
                            if c == nch // 2 and qb < NS - 1:
                                # one V super-block for qb+1 per pair
                                emit_vblock((qb + 1) * 4 + p * 2)
                        for lag in lagged:
                            emit_ctx(lag)

                        # denominator rows -> two independent [1,512]
                        # tiles so the copies run in parallel (even on
                        # DVE, odd on ACT) and nothing serializes the
                        # broadcast matmuls at the next pair boundary
                        den_e = smp.tile([1, 512], BF16, tag="den", name="den_e")
                        den_o = smp.tile([1, 512], BF16, tag="deno", name="den_o")
                        nc.vector.tensor_copy(
                            out=den_e[:], in_=ctx_e[64:65, :]
                        )
                        nc.scalar.copy(
                            out=den_o[:], in_=ctx_o[0:1, :]
                        )
                        pending = (ctx_e, ctx_o, den_e, den_o, p, qb)

                emit_norm_pair(*pending)
                pending = None
                for s16, do in slots:
                    emit_op_slot(s16, do)
                for s16 in range((NS - 1) * 4, NS * 4):
                    emit_op_slot(s16, 0)
                    emit_op_slot(s16, 1)
    if not nc.is_finalized():
        nc.finalize()
    return nc


def _prep_inputs(embeddings, Wq, bq, Wk, bk, Wv, bv, Wo, bo):
    embeddings = np.asarray(embeddings, np.float32)
    Wq, bq = np.asarray(Wq, np.float32), np.asarray(bq, np.float32)
    Wk = np.asarray(Wk, np.float32)
    Wv, bv = np.asarray(Wv, np.float32), np.asarray(bv, np.float32)
    Wo = np.asarray(Wo, np.float32)

    import ml_dtypes
    bf16_t = ml_dtypes.bfloat16
    # mask01[p, j] = 1 where col j (query) >= partition p (key) in the
    # 128x128 diagonal block, else 0
    mask01 = np.triu(np.ones((128, 128), np.float32)).astype(bf16_t)
    vfix = np.zeros((128, 64), np.float32)
    # two ones columns: va col 64 puts the denominator on partition 64 for
    # even heads / partition 0 for odd heads; va col 65 duplicates it on
    # partition 1 so the odd-head den copy needs no partition shift
    vfix[:, 0] = 1.0
    vfix[:, 1] = 1.0
    vfix = vfix.astype(bf16_t)
    sel = np.zeros((2, 128), np.float32)
    sel[0, 0:64] = 1.0
    sel[1, 64:128] = 1.0
    sel = sel.astype(bf16_t)

    in_maps = []
    for c in range(NCORES):
        b, g = c // 4, c % 4
        hs = HPC * g
        # [128, ND, S]: partition = d % 128, chunk = d // 128
        xt = np.ascontiguousarray(
            embeddings[b].T.reshape(ND, 128, S).transpose(1, 0, 2)
        ).astype(bf16_t)
        # 1/sqrt(dk) folded into Wq/bq (exact power of two)
        wq2 = np.stack(
            [np.concatenate([Wq[hs + 2 * p], Wq[hs + 2 * p + 1]], axis=1)
             for p in range(NPAIR)]
        ) * 0.125
        wk2 = np.stack(
            [np.concatenate([Wk[hs + 2 * p], Wk[hs + 2 * p + 1]], axis=1)
             for p in range(NPAIR)]
        )
        # [NPAIR, D, 128] -> [128, NPAIR, ND, 128] single-DMA layout
        wq2 = wq2.reshape(NPAIR, ND, 128, 128).transpose(2, 0, 1, 3)
        wk2 = wk2.reshape(NPAIR, ND, 128, 128).transpose(2, 0, 1, 3)
        wv4 = np.concatenate([Wv[hs + h] for h in range(HPC)], axis=1)
        wv4 = wv4.reshape(ND, 128, 256).transpose(1, 0, 2)
        wo4 = Wo[hs * DK:(hs + HPC) * DK, :].reshape(2, 128, D).transpose(1, 0, 2)
        bq2 = np.stack(
            [np.concatenate([bq[hs + 2 * p], bq[hs + 2 * p + 1]]) / 8.0
             for p in range(NPAIR)], axis=1
        )
        bvb = np.zeros((128, NPAIR, 2, 64), np.float32)
        for p in range(NPAIR):
            bvb[:, p, 0, :] = bv[hs + 2 * p][None, :]
            bvb[:, p, 1, :] = bv[hs + 2 * p + 1][None, :]
        in_maps.append({
            "xt": xt,
            "wq": np.ascontiguousarray(wq2).astype(bf16_t),
            "wk": np.ascontiguousarray(wk2).astype(bf16_t),
            "wv": np.ascontiguousarray(wv4).astype(bf16_t),
            "wo": np.ascontiguousarray(wo4).astype(bf16_t),
            "bq": np.ascontiguousarray(bq2),
            "bv_bc": bvb,
            "mask01": mask01,
            "sel": sel,
            "vfix": vfix,
        })
    return in_maps


def kernel(embeddings, Wq, bq, Wk, bk, Wv, bv, Wo, bo, _trace=False, _trace_kw=None):
    if "nc" not in _CACHE:
        _CACHE["nc"] = _build_bass()
    nc = _CACHE["nc"]
    in_maps = _prep_inputs(embeddings, Wq, bq, Wk, bk, Wv, bv, Wo, bo)
    kw = dict(_trace_kw or {})
    res = run_bass_kernel_spmd(
        nc, in_maps, core_ids=list(range(NCORES)), trace=_trace, **kw
    )
    _CACHE["last_result"] = res
    bo32 = np.asarray(bo, np.float32)
    out = np.empty((B, S, D), np.float32)
    for b in range(B):
        acc = np.asarray(res.results[4 * b]["out"], np.float32).copy()
        for g in range(1, 4):
            acc += np.asarray(res.results[4 * b + g]["out"], np.float32)
        out[b] = acc + bo32
    return out


# revision 41
# speedup vs baseline: 1.0629x; 1.0053x over previous
"""Causal multi-head attention on 8 trn2 NeuronCores.

Sharding: core c handles batch b=c//4 and heads [4*(c%4), 4*(c%4)+4).
Each core computes its 4 heads' attention plus the partial output
projection against the matching 256 rows of Wo; the host sums the 4
partials per batch (the all-reduce implied by row-sharding Wo) and adds
bo.

v11 (on top of v5's bf16 matmuls / fused biases / denominator-column
softmax / lagged ctx pipeline). Measured ~172-177us vs the 204.8us v5
baseline (run-to-run spread is DVFS throttling, ~0.78-0.82 avg util
limit in the NTFF counters):
  - Input DMA rebuilt: weights packed host-side so each tensor lands in
    one large HW-DGE transfer (2-4KB per partition line). Each queue is
    latency-bound near ~130-150GB/s while all 8 cores load their inputs
    (chip-level HBM contention), so xt streams across all THREE queues
    in demand order: chunks 0-1 scalar-HW, 2-3 gpsimd-SW, 4-7 sync-HW
    behind the pair-0 weights; pair-1 weights and Wv/Wo trail.
  - Q/K projections run c-outer/sb-inner (8 PSUM banks: 4 Q + 4 K per
    pair) so the PE consumes xt chunk-by-chunk right behind the DMA.
  - The two heads of a pair run TOGETHER through attention, chunk by
    chunk: their scores share one 2-bank PSUM tile, one rectangular
    strided ACT instruction exponentiates both halves with zero wasted
    columns (ACTIVATE's ~340ns fixed cost made one-exp-per-chunk the
    attention-phase limiter), one DVE op masks both causal triangles
    (replacing v5's identity-matmul -1e9 adds: ~20k PE columns + their
    LDWEIGHTS), and the pair's softmax normalization shares a single
    broadcast + reciprocal. Masking precedes the ones-column ctx
    matmul, so the denominator stays exact.
  - Denominator broadcast split into two rank-1 sel-matmuls whose den
    rows are copied in parallel (DVE + ACT) so nothing serializes the
    pair-boundary norm chain; ctx trails its exp by four chunks.
  - Output-projection slots for the previous query block are spread
    evenly through the score/ctx stream (the PE fills the gaps the exp
    pipeline leaves), evictions alternate ACT/DVE, and stores alternate
    the sync/scalar DMA queues. Output is fp16 (halves the 8MB/core
    store; fp16 beats bf16 on precision at this scale), one [128,1024]
    DMA per row block.
"""

import sys

for _p in ("/opt/trn_rl_repo", "/root/.axon_site/_ro/trn_rl_repo"):
    if _p not in sys.path:
        sys.path.insert(0, _p)

import numpy as np

import concourse.bass as bass
import concourse.bacc as bacc
import concourse.tile as tile
from concourse import mybir
from concourse.bass_utils import run_bass_kernel_spmd

F32 = mybir.dt.float32
F16 = mybir.dt.float16
BF16 = mybir.dt.bfloat16

B, S, D, H, DK = 2, 2048, 1024, 16, 64
NCORES = 8
HPC = 4          # heads per core
NPAIR = 2        # head pairs per core
ND = D // 128    # 8 contraction chunks over d
NS = S // 512    # 4 query blocks
NS16 = S // 128  # 16 sequence chunks

_CACHE = {}


def _build_bass():
    nc = bacc.Bacc(None)
    xt = nc.dram_tensor("xt", [128, ND, S], BF16, kind="ExternalInput")
    wq = nc.dram_tensor("wq", [128, NPAIR, ND, 128], BF16, kind="ExternalInput")
    wk = nc.dram_tensor("wk", [128, NPAIR, ND, 128], BF16, kind="ExternalInput")
    wv = nc.dram_tensor("wv", [128, ND, 256], BF16, kind="ExternalInput")
    wo = nc.dram_tensor("wo", [128, 2, D], BF16, kind="ExternalInput")
    bq = nc.dram_tensor("bq", [128, NPAIR], F32, kind="ExternalInput")
    bv_bc = nc.dram_tensor("bv_bc", [128, NPAIR, 2, 64], F32, kind="ExternalInput")
    mask01 = nc.dram_tensor("mask01", [128, 128], BF16, kind="ExternalInput")
    sel = nc.dram_tensor("sel", [2, 128], BF16, kind="ExternalInput")
    vfix = nc.dram_tensor("vfix", [128, 64], BF16, kind="ExternalInput")
    out = nc.dram_tensor("out", [S, D], F16, kind="ExternalOutput")

    with nc.allow_low_precision("bf16 operands; accumulation stays fp32 in PSUM"), \
            tile.TileContext(nc) as tc:
        with (
            tc.tile_pool(name="consts", bufs=1) as consts,
            tc.tile_pool(name="qkv", bufs=1) as qkv,
        ):
            wq_sb = consts.tile([128, NPAIR, ND, 128], BF16, tag="wq")
            wk_sb = consts.tile([128, NPAIR, ND, 128], BF16, tag="wk")
            wv_sb = consts.tile([128, ND, 256], BF16, tag="wv")
            wo_sb = consts.tile([128, 2, D], BF16, tag="wo")
            bq_sb = consts.tile([128, NPAIR], F32, tag="bq")
            bv_sb = consts.tile([128, NPAIR, 2, 64], F32, tag="bv")
            mask_sb = consts.tile([128, 128], BF16, tag="mask01")
            sela_sb = consts.tile([1, 128], BF16, tag="sela")
            selb_sb = consts.tile([1, 128], BF16, tag="selb")

            qt_sb = qkv.tile([128, NPAIR, S], BF16, tag="qt")
            kt_sb = qkv.tile([128, NPAIR, S], BF16, tag="kt")
            # Vaug per pair: cols 0:64 V_even | 64 ones | 65:128 zeros
            # | 128:192 V_odd. Even lhsT = cols 0:65 -> ctx on parts
            # 0:64 (+denominator row 64); odd lhsT = cols 64:192 ->
            # denominator on part 0, ctx on parts 64:128.
            va_sb = qkv.tile([128, NPAIR, NS16, 192], BF16, tag="va")
            ctxcat_sb = qkv.tile([128, 2, S], BF16, tag="ctxcat")

            # xt lives in the outer pool: the V projection now runs
            # interleaved with the attention phase and reads it there
            xt_sb = qkv.tile([128, ND, S], BF16, tag="xt")
            with (
                tc.tile_pool(name="mmp", bufs=8, space="PSUM") as mmp,
            ):
                # xt per-chunk on the scalar HW-DGE queue, weights as one
                # large DMA each on the sync HW queue, tiny consts on the
                # gpsimd SW queue: three queues stream in parallel and
                # every HW transfer moves 2-4KB per partition line.
                nc.scalar.dma_start(out=xt_sb[:, 0, :], in_=xt[:, 0, :])
                nc.scalar.dma_start(out=xt_sb[:, 1, :], in_=xt[:, 1, :])
                nc.gpsimd.dma_start(out=xt_sb[:, 2, :], in_=xt[:, 2, :])
                nc.gpsimd.dma_start(out=xt_sb[:, 3, :], in_=xt[:, 3, :])
                nc.sync.dma_start(out=wq_sb[:, 0], in_=wq[:, 0])
                nc.sync.dma_start(out=wk_sb[:, 0], in_=wk[:, 0])
                nc.gpsimd.dma_start(out=bq_sb[:], in_=bq[:])
                for c in range(4, ND):
                    nc.sync.dma_start(out=xt_sb[:, c, :], in_=xt[:, c, :])
                nc.sync.dma_start(out=wq_sb[:, 1], in_=wq[:, 1])
                nc.sync.dma_start(out=wk_sb[:, 1], in_=wk[:, 1])
                nc.sync.dma_start(out=wv_sb[:], in_=wv[:])
                nc.sync.dma_start(out=wo_sb[:], in_=wo[:])
                nc.gpsimd.dma_start(out=bv_sb[:], in_=bv_bc[:])
                nc.gpsimd.dma_start(out=mask_sb[:], in_=mask01[:])
                nc.gpsimd.dma_start(out=sela_sb[:], in_=sel[0:1, :])
                nc.gpsimd.dma_start(out=selb_sb[:], in_=sel[1:2, :])
                for p in range(NPAIR):
                    vfix_bc = bass.AP(
                        tensor=vfix.ap().tensor,
                        offset=0,
                        ap=[[64, 128], [0, NS16], [1, 64]],
                    )
                    nc.gpsimd.dma_start(out=va_sb[:, p, :, 64:128], in_=vfix_bc)

                # ---- Q^T / K^T projections (per pair, dk on partitions).
                # c-outer so the PE wants xt chunk c only ~1.7us after
                # chunk c-1: it trails right behind the streaming DMA.
                for p in range(NPAIR):
                    qps = [
                        mmp.tile([128, 512], F32, tag="mm", name=f"qp{sb}")
                        for sb in range(NS)
                    ]
                    kps = [
                        mmp.tile([128, 512], F32, tag="mm", name=f"kp{sb}")
                        for sb in range(NS)
                    ]
                    for c in range(ND):
                        for sb in range(NS):
                            nc.tensor.matmul(
                                qps[sb][:],
                                lhsT=wq_sb[:, p, c, :],
                                rhs=xt_sb[:, c, sb * 512:(sb + 1) * 512],
                                start=(c == 0),
                                stop=(c == ND - 1),
                            )
                        for sb in range(NS):
                            nc.tensor.matmul(
                                kps[sb][:],
                                lhsT=wk_sb[:, p, c, :],
                                rhs=xt_sb[:, c, sb * 512:(sb + 1) * 512],
                                start=(c == 0),
                                stop=(c == ND - 1),
                            )
                    for sb in range(NS):
                        nc.scalar.activation(
                            out=qt_sb[:, p, sb * 512:(sb + 1) * 512],
                            in_=qps[sb][:],
                            func=mybir.ActivationFunctionType.Identity,
                            bias=bq_sb[:, p:p + 1],
                            scale=1.0,
                        )
                        nc.vector.tensor_copy(
                            out=kt_sb[:, p, sb * 512:(sb + 1) * 512],
                            in_=kps[sb][:],
                        )

            # ---- attention + output projection, per query block.
            # The two heads of a pair run TOGETHER chunk-by-chunk: their
            # scores share one 2-bank PSUM tile (even head bank 0, odd
            # bank 1), one rectangular strided ACT instruction
            # exponentiates both with zero wasted columns, one DVE op
            # masks both diagonal triangles, and the pair's softmax
            # normalization shares a single sel-matmul broadcast +
            # reciprocal. Output-projection slots for the previous qb are
            # spread evenly through the score/ctx stream so the PE fills
            # the gaps that the exp pipeline leaves.
            with (
                tc.tile_pool(name="stp", bufs=2, space="PSUM") as stp,
                tc.tile_pool(name="ctxp", bufs=2, space="PSUM") as ctxp,
                tc.tile_pool(name="vpp", bufs=1, space="PSUM") as vpp,
                tc.tile_pool(name="ptp", bufs=6) as ptp,
                tc.tile_pool(name="smp", bufs=3) as smp,
                tc.tile_pool(name="outp", bufs=3) as outp,
            ):
                def emit_vblock(s0):
                    # V projection super-block (two seq row-blocks in one
                    # PSUM bank), in natural [s, dk] layout, 4 heads at
                    # once. Interleaved into the attention stream: qb's
                    # ctx only needs va rows < (qb+1)*4, so block s0 for
                    # qb+1 rides inside qb's score/ctx stream, filling
                    # the PE gaps the exp pipeline leaves. bv is added
                    # during the eviction (tensor_add with a
                    # partition-broadcast constant): exact through the
                    # softmax denominator trick since rows of P sum to
                    # den.
                    vp = vpp.tile([128, 2, 256], F32, tag="vp", name="vp")
                    for bi in range(2):
                        s16 = s0 + bi
                        for c in range(ND):
                            nc.tensor.matmul(
                                vp[:, bi, :],
                                lhsT=xt_sb[:, c, s16 * 128:(s16 + 1) * 128],
                                rhs=wv_sb[:, c, :],
                                start=(c == 0),
                                stop=(c == ND - 1),
                            )
                        # V_even -> va cols 0:64, V_odd -> cols 128:192
                        # in one two-segment add per pair
                        for p2 in range(NPAIR):
                            d0 = va_sb[:, p2, s16, 0:64]
                            dst = bass.AP(
                                tensor=d0.tensor, offset=d0.offset,
                                ap=[[d0.ap[0][0], 128], [128, 2], [1, 64]],
                            )
                            s0v = vp[:, bi, p2 * 128:(p2 + 1) * 128]
                            srcv = bass.AP(
                                tensor=s0v.tensor, offset=s0v.offset,
                                ap=[[s0v.ap[0][0], 128], [64, 2], [1, 64]],
                            )
                            nc.vector.tensor_add(
                                out=dst, in0=srcv, in1=bv_sb[:, p2, :, :]
                            )
                def emit_norm_pair(ctx_e, ctx_o, den_e, den_o, p, qb):
                    # two rank-1 broadcast matmuls serve both heads: sel_a
                    # routes den_e to partitions 0:64, sel_b routes den_o
                    # to 64:128, matching the ctx parity layout. The two
                    # den copies live in separate tiles so DVE and ACT
                    # produce them in parallel. custom-DVE ops (and
                    # tile_position=(0,64) matmuls) misbehave on HW when
                    # based at partition 64, so everything stays at base 0.
                    bc_ps = ctxp.tile([128, 512], F32, tag="op", name="bc_ps", bufs=1)
                    nc.tensor.matmul(
                        bc_ps[:],
                        lhsT=sela_sb[:],
                        rhs=den_e[:],
                        start=True,
                        stop=False,
                    )
                    nc.tensor.matmul(
                        bc_ps[:],
                        lhsT=selb_sb[:],
                        rhs=den_o[:],
                        start=False,
                        stop=True,
                    )
                    rcp = smp.tile([128, 512], F32, tag="rcp", name="rcp")
                    nc.vector.reciprocal_approx_fast(out=rcp[:], in_=bc_ps[:])
                    nc.vector.tensor_mul(
                        out=ctxcat_sb[0:64, p, qb * 512:(qb + 1) * 512],
                        in0=ctx_e[0:64, :],
                        in1=rcp[0:64, :],
                    )
                    nc.vector.tensor_mul(
                        out=ctxcat_sb[64:128, p, qb * 512:(qb + 1) * 512],
                        in0=ctx_o[64:128, :],
                        in1=rcp[64:128, :],
                    )

                ot_tiles = {}

                def emit_op_slot(s16, do):
                    # one (row-block, output-half) slice of the output
                    # projection: two accumulating matmuls, an eviction,
                    # and (on the second half) the store
                    if do == 0:
                        ot_tiles[s16] = outp.tile(
                            [128, D], F16, tag="ot", name="ot"
                        )
                    ot = ot_tiles[s16]
                    op = ctxp.tile([128, 512], F32, tag="op", name="op", bufs=1)
                    nc.tensor.matmul(
                        op[:],
                        lhsT=ctxcat_sb[:, 0, s16 * 128:(s16 + 1) * 128],
                        rhs=wo_sb[:, 0, do * 512:(do + 1) * 512],
                        start=True,
                        stop=False,
                    )
                    nc.tensor.matmul(
                        op[:],
                        lhsT=ctxcat_sb[:, 1, s16 * 128:(s16 + 1) * 128],
                        rhs=wo_sb[:, 1, do * 512:(do + 1) * 512],
                        start=False,
                        stop=True,
                    )
                    if do == 0:
                        # both halves evict on DVE: ACT is the pace-setter
                        # of the long score sweeps these slots ride in
                        nc.vector.tensor_copy(out=ot[:, 0:512], in_=op[:])
                    else:
                        nc.vector.tensor_copy(out=ot[:, 512:1024], in_=op[:])
                        oq = nc.sync if s16 % 2 == 0 else nc.scalar
                        oq.dma_start(
                            out=out[s16 * 128:(s16 + 1) * 128, :], in_=ot[:]
                        )

                emit_vblock(0)
                pending = None  # (ctx_e, ctx_o, den_e, den_o, p, qb)
                slots = []  # outproj work carried across query blocks
                for qb in range(NS):
                    nch = (qb + 1) * 4
                    if qb > 0:
                        slots += [(s16, do)
                                  for s16 in range((qb - 1) * 4, qb * 4)
                                  for do in range(2)]
                    # later query blocks have longer ACT-gated score
                    # sweeps and no V filler, so save outproj slots for
                    # them: drain ~6 per middle qb, everything in the last
                    want = len(slots) if qb == NS - 1 else min(len(slots), 6)
                    stride = max(1, (2 * nch) // want) if want else 1
                    drained = 0
                    for p in range(NPAIR):
                        qs_e = qt_sb[0:64, p, qb * 512:(qb + 1) * 512]
                        qs_o = qt_sb[64:128, p, qb * 512:(qb + 1) * 512]
                        ctx_e = ctxp.tile([128, 512], F32, tag="ctx", name="ctx_e")
                        ctx_o = ctxp.tile([128, 512], F32, tag="ctx", name="ctx_o")
                        lagged = []  # (pt, f0, diag, c) awaiting ctx mms

                        def emit_ctx(lag, ctx_e=ctx_e, ctx_o=ctx_o, p=p, nch=nch):
                            pt, f0, diag, c = lag
                            oe = ctx_e[0:65, f0:512] if diag else ctx_e[0:65, :]
                            nc.tensor.matmul(
                                oe,
                                lhsT=va_sb[:, p, c, 0:65],
                                rhs=pt[:, f0:512],
                                start=(c == 0),
                                stop=(c == nch - 1),
                            )
                            oo = ctx_o[:, f0:512] if diag else ctx_o[:]
                            nc.tensor.matmul(
                                oo,
                                lhsT=va_sb[:, p, c, 64:192],
                                rhs=pt[:, 512 + f0:1024],
                                start=(c == 0),
                                stop=(c == nch - 1),
                            )

                        for c in range(nch):
                            st = stp.tile([128, 1024], F32, tag="st", name="st")
                            pt = ptp.tile([128, 1024], BF16, tag="pt", name="pt")
                            diag = c >= qb * 4
                            # columns [0, f0) of a diag block are fully
                            # masked (q < kv everywhere): skip them.
                            f0 = 128 * (c - qb * 4) if diag else 0
                            nc.tensor.matmul(
                                st[:, f0:512],
                                lhsT=kt_sb[0:64, p, c * 128:(c + 1) * 128],
                                rhs=qs_e[:, f0:512],
                                start=True,
                                stop=True,
                            )
                            nc.tensor.matmul(
                                st[:, 512 + f0:1024],
                                lhsT=kt_sb[64:128, p, c * 128:(c + 1) * 128],
                                rhs=qs_o[:, f0:512],
                                start=True,
                                stop=True,
                            )
                            # one exp over both heads' halves: rectangular
                            # strided AP, no garbage columns
                            sv = st[:, f0:512]
                            pv = pt[:, f0:512]
                            nc.scalar.activation(
                                out=bass.AP(
                                    tensor=pv.tensor, offset=pv.offset,
                                    ap=[[pv.ap[0][0], 128], [512, 2],
                                        [1, 512 - f0]],
                                ),
                                in_=bass.AP(
                                    tensor=sv.tensor, offset=sv.offset,
                                    ap=[[sv.ap[0][0], 128], [512, 2],
                                        [1, 512 - f0]],
                                ),
                                func=mybir.ActivationFunctionType.Exp,
                            )
                            if diag:
                                # zero both heads' masked triangles in one
                                # DVE op; masking precedes the ones-column
                                # ctx matmul so the denominator stays exact
                                blk = pt[:, f0:f0 + 128]
                                mdst = bass.AP(
                                    tensor=blk.tensor, offset=blk.offset,
                                    ap=[[blk.ap[0][0], 128], [512, 2],
                                        [1, 128]],
                                )
                                msk = bass.AP(
                                    tensor=mask_sb.tensor,
                                    offset=mask_sb.offset,
                                    ap=[[mask_sb.ap[0][0], 128], [0, 2],
                                        [1, 128]],
                                )
                                nc.vector.tensor_mul(
                                    out=mdst, in0=mdst, in1=msk
                                )
                            # ctx trails four chunks behind its exp so the
                            # PE queue stays deep (hides ACT latency,
                            # semaphore propagation, and LDWEIGHTS)
                            lagged.append((pt, f0, diag, c))
                            while len(lagged) > 4:
                                emit_ctx(lagged.pop(0))
                            nf = 2 if nch <= 4 else 4
                            if c == nf and pending is not None:
                                emit_norm_pair(*pending)
                                pending = None
                            g = p * nch + c
                            if (drained < want and g >= 5
                                    and (g - 5) % stride == 0):
                                emit_op_slot(*slots.pop(0))
                                drained += 1
                            if qb == 0 and p == 0 and c == 1:
                                # second half of qb0's V rides inside its
                                # own score stream (fills the wait for the
                                # first super-block's DVE evictions)
                                emit_vblock(2)
                            if c == nch // 2 and qb < NS - 1:
                                # one V super-block for qb+1 per pair
                                emit_vblock((qb + 1) * 4 + p * 2)
                        for lag in lagged:
                            emit_ctx(lag)

                        # denominator rows -> two independent [1,512]
                        # tiles so the copies run in parallel (even on
                        # DVE, odd on ACT) and nothing serializes the
                        # broadcast matmuls at the next pair boundary
                        den_e = smp.tile([1, 512], BF16, tag="den", name="den_e")
                        den_o = smp.tile([1, 512], BF16, tag="deno", name="den_o")
                        nc.vector.tensor_copy(
                            out=den_e[:], in_=ctx_e[64:65, :]
                        )
                        nc.vector.tensor_copy(
                            out=den_o[:], in_=ctx_o[0:1, :]
                        )
                        pending = (ctx_e, ctx_o, den_e, den_o, p, qb)

                emit_norm_pair(*pending)
                pending = None
                for s16, do in slots:
                    emit_op_slot(s16, do)
                for s16 in range((NS - 1) * 4, NS * 4):
                    emit_op_slot(s16, 0)
                    emit_op_slot(s16, 1)
    if not nc.is_finalized():
        nc.finalize()
    return nc


def _prep_inputs(embeddings, Wq, bq, Wk, bk, Wv, bv, Wo, bo):
    embeddings = np.asarray(embeddings, np.float32)
    Wq, bq = np.asarray(Wq, np.float32), np.asarray(bq, np.float32)
    Wk = np.asarray(Wk, np.float32)
    Wv, bv = np.asarray(Wv, np.float32), np.asarray(bv, np.float32)
    Wo = np.asarray(Wo, np.float32)

    import ml_dtypes
    bf16_t = ml_dtypes.bfloat16
    # mask01[p, j] = 1 where col j (query) >= partition p (key) in the
    # 128x128 diagonal block, else 0
    mask01 = np.triu(np.ones((128, 128), np.float32)).astype(bf16_t)
    vfix = np.zeros((128, 64), np.float32)
    # two ones columns: va col 64 puts the denominator on partition 64 for
    # even heads / partition 0 for odd heads; va col 65 duplicates it on
    # partition 1 so the odd-head den copy needs no partition shift
    vfix[:, 0] = 1.0
    vfix[:, 1] = 1.0
    vfix = vfix.astype(bf16_t)
    sel = np.zeros((2, 128), np.float32)
    sel[0, 0:64] = 1.0
    sel[1, 64:128] = 1.0
    sel = sel.astype(bf16_t)

    in_maps = []
    for c in range(NCORES):
        b, g = c // 4, c % 4
        hs = HPC * g
        # [128, ND, S]: partition = d % 128, chunk = d // 128
        xt = np.ascontiguousarray(
            embeddings[b].T.reshape(ND, 128, S).transpose(1, 0, 2)
        ).astype(bf16_t)
        # 1/sqrt(dk) folded into Wq/bq (exact power of two)
        wq2 = np.stack(
            [np.concatenate([Wq[hs + 2 * p], Wq[hs + 2 * p + 1]], axis=1)
             for p in range(NPAIR)]
        ) * 0.125
        wk2 = np.stack(
            [np.concatenate([Wk[hs + 2 * p], Wk[hs + 2 * p + 1]], axis=1)
             for p in range(NPAIR)]
        )
        # [NPAIR, D, 128] -> [128, NPAIR, ND, 128] single-DMA layout
        wq2 = wq2.reshape(NPAIR, ND, 128, 128).transpose(2, 0, 1, 3)
        wk2 = wk2.reshape(NPAIR, ND, 128, 128).transpose(2, 0, 1, 3)
        wv4 = np.concatenate([Wv[hs + h] for h in range(HPC)], axis=1)
        wv4 = wv4.reshape(ND, 128, 256).transpose(1, 0, 2)
        wo4 = Wo[hs * DK:(hs + HPC) * DK, :].reshape(2, 128, D).transpose(1, 0, 2)
        bq2 = np.stack(
            [np.concatenate([bq[hs + 2 * p], bq[hs + 2 * p + 1]]) / 8.0
             for p in range(NPAIR)], axis=1
        )
        bvb = np.zeros((128, NPAIR, 2, 64), np.float32)
        for p in range(NPAIR):
            bvb[:, p, 0, :] = bv[hs + 2 * p][None, :]
            bvb[:, p, 1, :] = bv[hs + 2 * p + 1][None, :]
        in_maps.append({
            "xt": xt,
            "wq": np.ascontiguousarray(wq2).astype(bf16_t),
            "wk": np.ascontiguousarray(wk2).astype(bf16_t),
            "wv": np.ascontiguousarray(wv4).astype(bf16_t),
            "wo": np.ascontiguousarray(wo4).astype(bf16_t),
            "bq": np.ascontiguousarray(bq2),
            "bv_bc": bvb,
            "mask01": mask01,
            "sel": sel,
            "vfix": vfix,
        })
    return in_maps


def kernel(embeddings, Wq, bq, Wk, bk, Wv, bv, Wo, bo, _trace=False, _trace_kw=None):
    if "nc" not in _CACHE:
        _CACHE["nc"] = _build_bass()
    nc = _CACHE["nc"]
    in_maps = _prep_inputs(embeddings, Wq, bq, Wk, bk, Wv, bv, Wo, bo)
    kw = dict(_trace_kw or {})
    res = run_bass_kernel_spmd(
        nc, in_maps, core_ids=list(range(NCORES)), trace=_trace, **kw
    )
    _CACHE["last_result"] = res
    bo32 = np.asarray(bo, np.float32)
    out = np.empty((B, S, D), np.float32)
    for b in range(B):
        acc = np.asarray(res.results[4 * b]["out"], np.float32).copy()
        for g in range(1, 4):
            acc += np.asarray(res.results[4 * b + g]["out"], np.float32)
        out[b] = acc + bo32
    return out


# revision 42
# speedup vs baseline: 1.0734x; 1.0099x over previous
"""Causal multi-head attention on 8 trn2 NeuronCores.

Sharding: core c handles batch b=c//4 and heads [4*(c%4), 4*(c%4)+4).
Each core computes its 4 heads' attention plus the partial output
projection against the matching 256 rows of Wo; the host sums the 4
partials per batch (the all-reduce implied by row-sharding Wo) and adds
bo.

v11 (on top of v5's bf16 matmuls / fused biases / denominator-column
softmax / lagged ctx pipeline). Measured ~172-177us vs the 204.8us v5
baseline (run-to-run spread is DVFS throttling, ~0.78-0.82 avg util
limit in the NTFF counters):
  - Input DMA rebuilt: weights packed host-side so each tensor lands in
    one large HW-DGE transfer (2-4KB per partition line). Each queue is
    latency-bound near ~130-150GB/s while all 8 cores load their inputs
    (chip-level HBM contention), so xt streams across all THREE queues
    in demand order: chunks 0-1 scalar-HW, 2-3 gpsimd-SW, 4-7 sync-HW
    behind the pair-0 weights; pair-1 weights and Wv/Wo trail.
  - Q/K projections run c-outer/sb-inner (8 PSUM banks: 4 Q + 4 K per
    pair) so the PE consumes xt chunk-by-chunk right behind the DMA.
  - The two heads of a pair run TOGETHER through attention, chunk by
    chunk: their scores share one 2-bank PSUM tile, one rectangular
    strided ACT instruction exponentiates both halves with zero wasted
    columns (ACTIVATE's ~340ns fixed cost made one-exp-per-chunk the
    attention-phase limiter), one DVE op masks both causal triangles
    (replacing v5's identity-matmul -1e9 adds: ~20k PE columns + their
    LDWEIGHTS), and the pair's softmax normalization shares a single
    broadcast + reciprocal. Masking precedes the ones-column ctx
    matmul, so the denominator stays exact.
  - Denominator broadcast split into two rank-1 sel-matmuls whose den
    rows are copied in parallel (DVE + ACT) so nothing serializes the
    pair-boundary norm chain; ctx trails its exp by four chunks.
  - Output-projection slots for the previous query block are spread
    evenly through the score/ctx stream (the PE fills the gaps the exp
    pipeline leaves), evictions alternate ACT/DVE, and stores alternate
    the sync/scalar DMA queues. Output is fp16 (halves the 8MB/core
    store; fp16 beats bf16 on precision at this scale), one [128,1024]
    DMA per row block.
"""

import sys

for _p in ("/opt/trn_rl_repo", "/root/.axon_site/_ro/trn_rl_repo"):
    if _p not in sys.path:
        sys.path.insert(0, _p)

import numpy as np

import concourse.bass as bass
import concourse.bacc as bacc
import concourse.tile as tile
from concourse import mybir
from concourse.bass_utils import run_bass_kernel_spmd

F32 = mybir.dt.float32
F16 = mybir.dt.float16
BF16 = mybir.dt.bfloat16

B, S, D, H, DK = 2, 2048, 1024, 16, 64
NCORES = 8
HPC = 4          # heads per core
NPAIR = 2        # head pairs per core
ND = D // 128    # 8 contraction chunks over d
NS = S // 512    # 4 query blocks
NS16 = S // 128  # 16 sequence chunks

_CACHE = {}


def _build_bass():
    nc = bacc.Bacc(None)
    xt = nc.dram_tensor("xt", [128, ND, S], BF16, kind="ExternalInput")
    wq = nc.dram_tensor("wq", [128, NPAIR, ND, 128], BF16, kind="ExternalInput")
    wk = nc.dram_tensor("wk", [128, NPAIR, ND, 128], BF16, kind="ExternalInput")
    wv = nc.dram_tensor("wv", [128, ND, 256], BF16, kind="ExternalInput")
    wo = nc.dram_tensor("wo", [128, 2, D], BF16, kind="ExternalInput")
    bq = nc.dram_tensor("bq", [128, NPAIR], F32, kind="ExternalInput")
    bv_bc = nc.dram_tensor("bv_bc", [128, NPAIR, 2, 64], F32, kind="ExternalInput")
    mask01 = nc.dram_tensor("mask01", [128, 128], BF16, kind="ExternalInput")
    sel = nc.dram_tensor("sel", [2, 128], BF16, kind="ExternalInput")
    vfix = nc.dram_tensor("vfix", [128, 64], BF16, kind="ExternalInput")
    out = nc.dram_tensor("out", [S, D], F16, kind="ExternalOutput")

    with nc.allow_low_precision("bf16 operands; accumulation stays fp32 in PSUM"), \
            tile.TileContext(nc) as tc:
        with (
            tc.tile_pool(name="consts", bufs=1) as consts,
            tc.tile_pool(name="qkv", bufs=1) as qkv,
        ):
            wq_sb = consts.tile([128, NPAIR, ND, 128], BF16, tag="wq")
            wk_sb = consts.tile([128, NPAIR, ND, 128], BF16, tag="wk")
            wv_sb = consts.tile([128, ND, 256], BF16, tag="wv")
            wo_sb = consts.tile([128, 2, D], BF16, tag="wo")
            bq_sb = consts.tile([128, NPAIR], F32, tag="bq")
            bv_sb = consts.tile([128, NPAIR, 2, 64], F32, tag="bv")
            mask_sb = consts.tile([128, 128], BF16, tag="mask01")
            sela_sb = consts.tile([1, 128], BF16, tag="sela")
            selb_sb = consts.tile([1, 128], BF16, tag="selb")

            qt_sb = qkv.tile([128, NPAIR, S], BF16, tag="qt")
            kt_sb = qkv.tile([128, NPAIR, S], BF16, tag="kt")
            # Vaug per pair: cols 0:64 V_even | 64 ones | 65:128 zeros
            # | 128:192 V_odd. Even lhsT = cols 0:65 -> ctx on parts
            # 0:64 (+denominator row 64); odd lhsT = cols 64:192 ->
            # denominator on part 0, ctx on parts 64:128.
            va_sb = qkv.tile([128, NPAIR, NS16, 192], BF16, tag="va")
            ctxcat_sb = qkv.tile([128, 2, S], BF16, tag="ctxcat")

            # xt lives in the outer pool: the V projection now runs
            # interleaved with the attention phase and reads it there
            xt_sb = qkv.tile([128, ND, S], BF16, tag="xt")
            with (
                tc.tile_pool(name="mmp", bufs=8, space="PSUM") as mmp,
            ):
                # xt per-chunk on the scalar HW-DGE queue, weights as one
                # large DMA each on the sync HW queue, tiny consts on the
                # gpsimd SW queue: three queues stream in parallel and
                # every HW transfer moves 2-4KB per partition line.
                nc.scalar.dma_start(out=xt_sb[:, 0, :], in_=xt[:, 0, :])
                nc.scalar.dma_start(out=xt_sb[:, 1, :], in_=xt[:, 1, :])
                nc.gpsimd.dma_start(out=xt_sb[:, 2, :], in_=xt[:, 2, :])
                nc.gpsimd.dma_start(out=xt_sb[:, 3, :], in_=xt[:, 3, :])
                nc.sync.dma_start(out=wq_sb[:, 0], in_=wq[:, 0])
                nc.sync.dma_start(out=wk_sb[:, 0], in_=wk[:, 0])
                nc.gpsimd.dma_start(out=bq_sb[:], in_=bq[:])
                for c in range(4, ND):
                    nc.sync.dma_start(out=xt_sb[:, c, :], in_=xt[:, c, :])
                nc.sync.dma_start(out=wq_sb[:, 1], in_=wq[:, 1])
                nc.sync.dma_start(out=wk_sb[:, 1], in_=wk[:, 1])
                nc.sync.dma_start(out=wv_sb[:], in_=wv[:])
                nc.sync.dma_start(out=wo_sb[:], in_=wo[:])
                nc.gpsimd.dma_start(out=bv_sb[:], in_=bv_bc[:])
                nc.gpsimd.dma_start(out=mask_sb[:], in_=mask01[:])
                nc.gpsimd.dma_start(out=sela_sb[:], in_=sel[0:1, :])
                nc.gpsimd.dma_start(out=selb_sb[:], in_=sel[1:2, :])
                for p in range(NPAIR):
                    vfix_bc = bass.AP(
                        tensor=vfix.ap().tensor,
                        offset=0,
                        ap=[[64, 128], [0, NS16], [1, 64]],
                    )
                    nc.gpsimd.dma_start(out=va_sb[:, p, :, 64:128], in_=vfix_bc)

                # ---- Q^T / K^T projections (per pair, dk on partitions).
                # c-outer so the PE wants xt chunk c only ~1.7us after
                # chunk c-1: it trails right behind the streaming DMA.
                for p in range(NPAIR):
                    qps = [
                        mmp.tile([128, 512], F32, tag="mm", name=f"qp{sb}")
                        for sb in range(NS)
                    ]
                    kps = [
                        mmp.tile([128, 512], F32, tag="mm", name=f"kp{sb}")
                        for sb in range(NS)
                    ]
                    for c in range(ND):
                        for sb in range(NS):
                            nc.tensor.matmul(
                                qps[sb][:],
                                lhsT=wq_sb[:, p, c, :],
                                rhs=xt_sb[:, c, sb * 512:(sb + 1) * 512],
                                start=(c == 0),
                                stop=(c == ND - 1),
                            )
                        for sb in range(NS):
                            nc.tensor.matmul(
                                kps[sb][:],
                                lhsT=wk_sb[:, p, c, :],
                                rhs=xt_sb[:, c, sb * 512:(sb + 1) * 512],
                                start=(c == 0),
                                stop=(c == ND - 1),
                            )
                    for sb in range(NS):
                        nc.scalar.activation(
                            out=qt_sb[:, p, sb * 512:(sb + 1) * 512],
                            in_=qps[sb][:],
                            func=mybir.ActivationFunctionType.Identity,
                            bias=bq_sb[:, p:p + 1],
                            scale=1.0,
                        )
                        nc.vector.tensor_copy(
                            out=kt_sb[:, p, sb * 512:(sb + 1) * 512],
                            in_=kps[sb][:],
                        )

            # ---- attention + output projection, per query block.
            # The two heads of a pair run TOGETHER chunk-by-chunk: their
            # scores share one 2-bank PSUM tile (even head bank 0, odd
            # bank 1), one rectangular strided ACT instruction
            # exponentiates both with zero wasted columns, one DVE op
            # masks both diagonal triangles, and the pair's softmax
            # normalization shares a single sel-matmul broadcast +
            # reciprocal. Output-projection slots for the previous qb are
            # spread evenly through the score/ctx stream so the PE fills
            # the gaps that the exp pipeline leaves.
            with (
                tc.tile_pool(name="stp", bufs=2, space="PSUM") as stp,
                tc.tile_pool(name="ctxp", bufs=2, space="PSUM") as ctxp,
                tc.tile_pool(name="vpp", bufs=1, space="PSUM") as vpp,
                tc.tile_pool(name="ptp", bufs=6) as ptp,
                tc.tile_pool(name="smp", bufs=3) as smp,
                tc.tile_pool(name="outp", bufs=3) as outp,
            ):
                def emit_vblock(s0):
                    # V projection super-block (two seq row-blocks in one
                    # PSUM bank), in natural [s, dk] layout, 4 heads at
                    # once. Interleaved into the attention stream: qb's
                    # ctx only needs va rows < (qb+1)*4, so block s0 for
                    # qb+1 rides inside qb's score/ctx stream, filling
                    # the PE gaps the exp pipeline leaves. bv is added
                    # during the eviction (tensor_add with a
                    # partition-broadcast constant): exact through the
                    # softmax denominator trick since rows of P sum to
                    # den.
                    vp = vpp.tile([128, 2, 256], F32, tag="vp", name="vp")
                    for bi in range(2):
                        s16 = s0 + bi
                        for c in range(ND):
                            nc.tensor.matmul(
                                vp[:, bi, :],
                                lhsT=xt_sb[:, c, s16 * 128:(s16 + 1) * 128],
                                rhs=wv_sb[:, c, :],
                                start=(c == 0),
                                stop=(c == ND - 1),
                            )
                        # V_even -> va cols 0:64, V_odd -> cols 128:192
                        # in one two-segment add per pair
                        for p2 in range(NPAIR):
                            d0 = va_sb[:, p2, s16, 0:64]
                            dst = bass.AP(
                                tensor=d0.tensor, offset=d0.offset,
                                ap=[[d0.ap[0][0], 128], [128, 2], [1, 64]],
                            )
                            s0v = vp[:, bi, p2 * 128:(p2 + 1) * 128]
                            srcv = bass.AP(
                                tensor=s0v.tensor, offset=s0v.offset,
                                ap=[[s0v.ap[0][0], 128], [64, 2], [1, 64]],
                            )
                            nc.vector.tensor_add(
                                out=dst, in0=srcv, in1=bv_sb[:, p2, :, :]
                            )
                def emit_norm_pair(ctx_e, ctx_o, den_e, den_o, p, qb):
                    # two rank-1 broadcast matmuls serve both heads: sel_a
                    # routes den_e to partitions 0:64, sel_b routes den_o
                    # to 64:128, matching the ctx parity layout. The two
                    # den copies live in separate tiles so DVE and ACT
                    # produce them in parallel. custom-DVE ops (and
                    # tile_position=(0,64) matmuls) misbehave on HW when
                    # based at partition 64, so everything stays at base 0.
                    bc_ps = ctxp.tile([128, 512], F32, tag="op", name="bc_ps", bufs=1)
                    nc.tensor.matmul(
                        bc_ps[:],
                        lhsT=sela_sb[:],
                        rhs=den_e[:],
                        start=True,
                        stop=False,
                    )
                    nc.tensor.matmul(
                        bc_ps[:],
                        lhsT=selb_sb[:],
                        rhs=den_o[:],
                        start=False,
                        stop=True,
                    )
                    rcp = smp.tile([128, 512], F32, tag="rcp", name="rcp")
                    nc.vector.reciprocal_approx_fast(out=rcp[:], in_=bc_ps[:])
                    nc.vector.tensor_mul(
                        out=ctxcat_sb[0:64, p, qb * 512:(qb + 1) * 512],
                        in0=ctx_e[0:64, :],
                        in1=rcp[0:64, :],
                    )
                    nc.vector.tensor_mul(
                        out=ctxcat_sb[64:128, p, qb * 512:(qb + 1) * 512],
                        in0=ctx_o[64:128, :],
                        in1=rcp[64:128, :],
                    )

                ot_tiles = {}

                def emit_op_slot(s16, do):
                    # one (row-block, output-half) slice of the output
                    # projection: two accumulating matmuls, an eviction,
                    # and (on the second half) the store
                    if do == 0:
                        ot_tiles[s16] = outp.tile(
                            [128, D], F16, tag="ot", name="ot"
                        )
                    ot = ot_tiles[s16]
                    op = ctxp.tile([128, 512], F32, tag="op", name="op", bufs=1)
                    nc.tensor.matmul(
                        op[:],
                        lhsT=ctxcat_sb[:, 0, s16 * 128:(s16 + 1) * 128],
                        rhs=wo_sb[:, 0, do * 512:(do + 1) * 512],
                        start=True,
                        stop=False,
                    )
                    nc.tensor.matmul(
                        op[:],
                        lhsT=ctxcat_sb[:, 1, s16 * 128:(s16 + 1) * 128],
                        rhs=wo_sb[:, 1, do * 512:(do + 1) * 512],
                        start=False,
                        stop=True,
                    )
                    if do == 0:
                        # both halves evict on DVE: ACT is the pace-setter
                        # of the long score sweeps these slots ride in
                        nc.vector.tensor_copy(out=ot[:, 0:512], in_=op[:])
                    else:
                        nc.vector.tensor_copy(out=ot[:, 512:1024], in_=op[:])
                        oq = nc.sync if s16 % 2 == 0 else nc.scalar
                        oq.dma_start(
                            out=out[s16 * 128:(s16 + 1) * 128, :], in_=ot[:]
                        )

                emit_vblock(0)
                pending = None  # (ctx_e, ctx_o, den_e, den_o, p, qb)
                slots = []  # outproj work carried across query blocks
                for qb in range(NS):
                    nch = (qb + 1) * 4
                    if qb > 0:
                        slots += [(s16, do)
                                  for s16 in range((qb - 1) * 4, qb * 4)
                                  for do in range(2)]
                    # later query blocks have longer ACT-gated score
                    # sweeps and no V filler, so save outproj slots for
                    # them: drain ~6 per middle qb, everything in the last
                    want = len(slots) if qb == NS - 1 else min(len(slots), 6)
                    stride = max(1, (2 * nch) // want) if want else 1
                    drained = 0
                    for p in range(NPAIR):
                        qs_e = qt_sb[0:64, p, qb * 512:(qb + 1) * 512]
                        qs_o = qt_sb[64:128, p, qb * 512:(qb + 1) * 512]
                        ctx_e = ctxp.tile([128, 512], F32, tag="ctx", name="ctx_e")
                        ctx_o = ctxp.tile([128, 512], F32, tag="ctx", name="ctx_o")
                        lagged = []  # (pt, f0, diag, c) awaiting ctx mms

                        def emit_ctx(lag, ctx_e=ctx_e, ctx_o=ctx_o, p=p, nch=nch):
                            pt, f0, diag, c = lag
                            oe = ctx_e[0:65, f0:512] if diag else ctx_e[0:65, :]
                            nc.tensor.matmul(
                                oe,
                                lhsT=va_sb[:, p, c, 0:65],
                                rhs=pt[:, f0:512],
                                start=(c == 0),
                                stop=(c == nch - 1),
                            )
                            oo = ctx_o[:, f0:512] if diag else ctx_o[:]
                            nc.tensor.matmul(
                                oo,
                                lhsT=va_sb[:, p, c, 64:192],
                                rhs=pt[:, 512 + f0:1024],
                                start=(c == 0),
                                stop=(c == nch - 1),
                            )

                        for c in range(nch):
                            st = stp.tile([128, 1024], F32, tag="st", name="st")
                            pt = ptp.tile([128, 1024], BF16, tag="pt", name="pt")
                            diag = c >= qb * 4
                            # columns [0, f0) of a diag block are fully
                            # masked (q < kv everywhere): skip them.
                            f0 = 128 * (c - qb * 4) if diag else 0
                            nc.tensor.matmul(
                                st[:, f0:512],
                                lhsT=kt_sb[0:64, p, c * 128:(c + 1) * 128],
                                rhs=qs_e[:, f0:512],
                                start=True,
                                stop=True,
                            )
                            nc.tensor.matmul(
                                st[:, 512 + f0:1024],
                                lhsT=kt_sb[64:128, p, c * 128:(c + 1) * 128],
                                rhs=qs_o[:, f0:512],
                                start=True,
                                stop=True,
                            )
                            # one exp over both heads' halves: rectangular
                            # strided AP, no garbage columns
                            sv = st[:, f0:512]
                            pv = pt[:, f0:512]
                            nc.scalar.activation(
                                out=bass.AP(
                                    tensor=pv.tensor, offset=pv.offset,
                                    ap=[[pv.ap[0][0], 128], [512, 2],
                                        [1, 512 - f0]],
                                ),
                                in_=bass.AP(
                                    tensor=sv.tensor, offset=sv.offset,
                                    ap=[[sv.ap[0][0], 128], [512, 2],
                                        [1, 512 - f0]],
                                ),
                                func=mybir.ActivationFunctionType.Exp,
                            )
                            if diag:
                                # zero both heads' masked triangles in one
                                # DVE op; masking precedes the ones-column
                                # ctx matmul so the denominator stays exact
                                blk = pt[:, f0:f0 + 128]
                                mdst = bass.AP(
                                    tensor=blk.tensor, offset=blk.offset,
                                    ap=[[blk.ap[0][0], 128], [512, 2],
                                        [1, 128]],
                                )
                                msk = bass.AP(
                                    tensor=mask_sb.tensor,
                                    offset=mask_sb.offset,
                                    ap=[[mask_sb.ap[0][0], 128], [0, 2],
                                        [1, 128]],
                                )
                                nc.vector.tensor_mul(
                                    out=mdst, in0=mdst, in1=msk
                                )
                            # ctx trails four chunks behind its exp so the
                            # PE queue stays deep (hides ACT latency,
                            # semaphore propagation, and LDWEIGHTS)
                            lagged.append((pt, f0, diag, c))
                            while len(lagged) > 4:
                                emit_ctx(lagged.pop(0))
                            nf = 2 if nch <= 4 else 4
                            if c == nf and pending is not None:
                                emit_norm_pair(*pending)
                                pending = None
                            g = p * nch + c
                            if (drained < want and g >= 5
                                    and (g - 5) % stride == 0):
                                emit_op_slot(*slots.pop(0))
                                drained += 1
                            if qb == 0 and p == 0 and c == 1:
                                # second half of qb0's V rides inside its
                                # own score stream (fills the wait for the
                                # first super-block's DVE evictions)
                                emit_vblock(2)
                            if c == nch // 2 and qb < NS - 1:
                                # one V super-block for qb+1 per pair
                                emit_vblock((qb + 1) * 4 + p * 2)
                        for lag in lagged:
                            emit_ctx(lag)

                        # denominator rows -> two independent [1,512]
                        # tiles so the copies run in parallel (even on
                        # DVE, odd on ACT) and nothing serializes the
                        # broadcast matmuls at the next pair boundary
                        den_e = smp.tile([1, 512], BF16, tag="den", name="den_e")
                        den_o = smp.tile([1, 512], BF16, tag="deno", name="den_o")
                        nc.vector.tensor_copy(
                            out=den_e[:], in_=ctx_e[64:65, :]
                        )
                        nc.scalar.copy(
                            out=den_o[:], in_=ctx_o[0:1, :]
                        )
                        pending = (ctx_e, ctx_o, den_e, den_o, p, qb)

                emit_norm_pair(*pending)
                pending = None
                for s16, do in slots:
                    emit_op_slot(s16, do)
                for s16 in range((NS - 1) * 4, NS * 4):
                    emit_op_slot(s16, 0)
                    emit_op_slot(s16, 1)
    if not nc.is_finalized():
        nc.finalize()
    return nc


def _prep_inputs(embeddings, Wq, bq, Wk, bk, Wv, bv, Wo, bo):
    embeddings = np.asarray(embeddings, np.float32)
    Wq, bq = np.asarray(Wq, np.float32), np.asarray(bq, np.float32)
    Wk = np.asarray(Wk, np.float32)
    Wv, bv = np.asarray(Wv, np.float32), np.asarray(bv, np.float32)
    Wo = np.asarray(Wo, np.float32)

    import ml_dtypes
    bf16_t = ml_dtypes.bfloat16
    # mask01[p, j] = 1 where col j (query) >= partition p (key) in the
    # 128x128 diagonal block, else 0
    mask01 = np.triu(np.ones((128, 128), np.float32)).astype(bf16_t)
    vfix = np.zeros((128, 64), np.float32)
    # two ones columns: va col 64 puts the denominator on partition 64 for
    # even heads / partition 0 for odd heads; va col 65 duplicates it on
    # partition 1 so the odd-head den copy needs no partition shift
    vfix[:, 0] = 1.0
    vfix[:, 1] = 1.0
    vfix = vfix.astype(bf16_t)
    sel = np.zeros((2, 128), np.float32)
    sel[0, 0:64] = 1.0
    sel[1, 64:128] = 1.0
    sel = sel.astype(bf16_t)

    in_maps = []
    for c in range(NCORES):
        b, g = c // 4, c % 4
        hs = HPC * g
        # [128, ND, S]: partition = d % 128, chunk = d // 128
        xt = np.ascontiguousarray(
            embeddings[b].T.reshape(ND, 128, S).transpose(1, 0, 2)
        ).astype(bf16_t)
        # 1/sqrt(dk) folded into Wq/bq (exact power of two)
        wq2 = np.stack(
            [np.concatenate([Wq[hs + 2 * p], Wq[hs + 2 * p + 1]], axis=1)
             for p in range(NPAIR)]
        ) * 0.125
        wk2 = np.stack(
            [np.concatenate([Wk[hs + 2 * p], Wk[hs + 2 * p + 1]], axis=1)
             for p in range(NPAIR)]
        )
        # [NPAIR, D, 128] -> [128, NPAIR, ND, 128] single-DMA layout
        wq2 = wq2.reshape(NPAIR, ND, 128, 128).transpose(2, 0, 1, 3)
        wk2 = wk2.reshape(NPAIR, ND, 128, 128).transpose(2, 0, 1, 3)
        wv4 = np.concatenate([Wv[hs + h] for h in range(HPC)], axis=1)
        wv4 = wv4.reshape(ND, 128, 256).transpose(1, 0, 2)
        wo4 = Wo[hs * DK:(hs + HPC) * DK, :].reshape(2, 128, D).transpose(1, 0, 2)
        bq2 = np.stack(
            [np.concatenate([bq[hs + 2 * p], bq[hs + 2 * p + 1]]) / 8.0
             for p in range(NPAIR)], axis=1
        )
        bvb = np.zeros((128, NPAIR, 2, 64), np.float32)
        for p in range(NPAIR):
            bvb[:, p, 0, :] = bv[hs + 2 * p][None, :]
            bvb[:, p, 1, :] = bv[hs + 2 * p + 1][None, :]
        in_maps.append({
            "xt": xt,
            "wq": np.ascontiguousarray(wq2).astype(bf16_t),
            "wk": np.ascontiguousarray(wk2).astype(bf16_t),
            "wv": np.ascontiguousarray(wv4).astype(bf16_t),
            "wo": np.ascontiguousarray(wo4).astype(bf16_t),
            "bq": np.ascontiguousarray(bq2),
            "bv_bc": bvb,
            "mask01": mask01,
            "sel": sel,
            "vfix": vfix,
        })
    return in_maps


def kernel(embeddings, Wq, bq, Wk, bk, Wv, bv, Wo, bo, _trace=False, _trace_kw=None):
    if "nc" not in _CACHE:
        _CACHE["nc"] = _build_bass()
    nc = _CACHE["nc"]
    in_maps = _prep_inputs(embeddings, Wq, bq, Wk, bk, Wv, bv, Wo, bo)
    kw = dict(_trace_kw or {})
    res = run_bass_kernel_spmd(
        nc, in_maps, core_ids=list(range(NCORES)), trace=_trace, **kw
    )
    _CACHE["last_result"] = res
    bo32 = np.asarray(bo, np.float32)
    out = np.empty((B, S, D), np.float32)
    for b in range(B):
        acc = np.asarray(res.results[4 * b]["out"], np.float32).copy()
        for g in range(1, 4):
            acc += np.asarray(res.results[4 * b + g]["out"], np.float32)
        out[b] = acc + bo32
    return out


# revision 43
# speedup vs baseline: 1.1077x; 1.0319x over previous
"""Causal multi-head attention on 8 trn2 NeuronCores.

Sharding: core c handles batch b=c//4 and heads [4*(c%4), 4*(c%4)+4).
Each core computes its 4 heads' attention plus the partial output
projection against the matching 256 rows of Wo; the host sums the 4
partials per batch (the all-reduce implied by row-sharding Wo) and adds
bo.

v11 (on top of v5's bf16 matmuls / fused biases / denominator-column
softmax / lagged ctx pipeline). Measured ~172-177us vs the 204.8us v5
baseline (run-to-run spread is DVFS throttling, ~0.78-0.82 avg util
limit in the NTFF counters):
  - Input DMA rebuilt: weights packed host-side so each tensor lands in
    one large HW-DGE transfer (2-4KB per partition line). Each queue is
    latency-bound near ~130-150GB/s while all 8 cores load their inputs
    (chip-level HBM contention), so xt streams across all THREE queues
    in demand order: chunks 0-1 scalar-HW, 2-3 gpsimd-SW, 4-7 sync-HW
    behind the pair-0 weights; pair-1 weights and Wv/Wo trail.
  - Q/K projections run c-outer/sb-inner (8 PSUM banks: 4 Q + 4 K per
    pair) so the PE consumes xt chunk-by-chunk right behind the DMA.
  - The two heads of a pair run TOGETHER through attention, chunk by
    chunk: their scores share one 2-bank PSUM tile, one rectangular
    strided ACT instruction exponentiates both halves with zero wasted
    columns (ACTIVATE's ~340ns fixed cost made one-exp-per-chunk the
    attention-phase limiter), one DVE op masks both causal triangles
    (replacing v5's identity-matmul -1e9 adds: ~20k PE columns + their
    LDWEIGHTS), and the pair's softmax normalization shares a single
    broadcast + reciprocal. Masking precedes the ones-column ctx
    matmul, so the denominator stays exact.
  - Denominator broadcast split into two rank-1 sel-matmuls whose den
    rows are copied in parallel (DVE + ACT) so nothing serializes the
    pair-boundary norm chain; ctx trails its exp by four chunks.
  - Output-projection slots for the previous query block are spread
    evenly through the score/ctx stream (the PE fills the gaps the exp
    pipeline leaves), evictions alternate ACT/DVE, and stores alternate
    the sync/scalar DMA queues. Output is fp16 (halves the 8MB/core
    store; fp16 beats bf16 on precision at this scale), one [128,1024]
    DMA per row block.
"""

import sys

for _p in ("/opt/trn_rl_repo", "/root/.axon_site/_ro/trn_rl_repo"):
    if _p not in sys.path:
        sys.path.insert(0, _p)

import numpy as np

import concourse.bass as bass
import concourse.bacc as bacc
import concourse.tile as tile
from concourse import mybir
from concourse.bass_utils import run_bass_kernel_spmd

F32 = mybir.dt.float32
F16 = mybir.dt.float16
BF16 = mybir.dt.bfloat16

B, S, D, H, DK = 2, 2048, 1024, 16, 64
NCORES = 8
HPC = 4          # heads per core
NPAIR = 2        # head pairs per core
ND = D // 128    # 8 contraction chunks over d
NS = S // 512    # 4 query blocks
NS16 = S // 128  # 16 sequence chunks

_CACHE = {}


def _build_bass():
    nc = bacc.Bacc(None)
    xt = nc.dram_tensor("xt", [128, ND, S], BF16, kind="ExternalInput")
    wq = nc.dram_tensor("wq", [128, NPAIR, ND, 128], BF16, kind="ExternalInput")
    wk = nc.dram_tensor("wk", [128, NPAIR, ND, 128], BF16, kind="ExternalInput")
    wv = nc.dram_tensor("wv", [128, ND, 256], BF16, kind="ExternalInput")
    wo = nc.dram_tensor("wo", [128, 2, D], BF16, kind="ExternalInput")
    bq = nc.dram_tensor("bq", [128, NPAIR], F32, kind="ExternalInput")
    bv_bc = nc.dram_tensor("bv_bc", [128, NPAIR, 2, 64], F32, kind="ExternalInput")
    mask01 = nc.dram_tensor("mask01", [128, 128], BF16, kind="ExternalInput")
    sel = nc.dram_tensor("sel", [2, 128], BF16, kind="ExternalInput")
    vfix = nc.dram_tensor("vfix", [128, 64], BF16, kind="ExternalInput")
    out = nc.dram_tensor("out", [S, D], F16, kind="ExternalOutput")

    with nc.allow_low_precision("bf16 operands; accumulation stays fp32 in PSUM"), \
            tile.TileContext(nc) as tc:
        with (
            tc.tile_pool(name="consts", bufs=1) as consts,
            tc.tile_pool(name="qkv", bufs=1) as qkv,
        ):
            wq_sb = consts.tile([128, NPAIR, ND, 128], BF16, tag="wq")
            wk_sb = consts.tile([128, NPAIR, ND, 128], BF16, tag="wk")
            wv_sb = consts.tile([128, ND, 256], BF16, tag="wv")
            wo_sb = consts.tile([128, 2, D], BF16, tag="wo")
            bq_sb = consts.tile([128, NPAIR], F32, tag="bq")
            bv_sb = consts.tile([128, NPAIR, 2, 64], F32, tag="bv")
            mask_sb = consts.tile([128, 128], BF16, tag="mask01")
            sela_sb = consts.tile([1, 128], BF16, tag="sela")
            selb_sb = consts.tile([1, 128], BF16, tag="selb")

            qt_sb = qkv.tile([128, NPAIR, S], BF16, tag="qt")
            kt_sb = qkv.tile([128, NPAIR, S], BF16, tag="kt")
            # Vaug per pair: cols 0:64 V_even | 64 ones | 65:128 zeros
            # | 128:192 V_odd. Even lhsT = cols 0:65 -> ctx on parts
            # 0:64 (+denominator row 64); odd lhsT = cols 64:192 ->
            # denominator on part 0, ctx on parts 64:128.
            va_sb = qkv.tile([128, NPAIR, NS16, 192], BF16, tag="va")
            ctxcat_sb = qkv.tile([128, 2, S], BF16, tag="ctxcat")

            # xt lives in the outer pool: the V projection now runs
            # interleaved with the attention phase and reads it there
            xt_sb = qkv.tile([128, ND, S], BF16, tag="xt")
            with (
                tc.tile_pool(name="mmp", bufs=8, space="PSUM") as mmp,
            ):
                # xt per-chunk on the scalar HW-DGE queue, weights as one
                # large DMA each on the sync HW queue, tiny consts on the
                # gpsimd SW queue: three queues stream in parallel and
                # every HW transfer moves 2-4KB per partition line.
                nc.scalar.dma_start(out=xt_sb[:, 0, :], in_=xt[:, 0, :])
                nc.scalar.dma_start(out=xt_sb[:, 1, :], in_=xt[:, 1, :])
                nc.gpsimd.dma_start(out=xt_sb[:, 2, :], in_=xt[:, 2, :])
                nc.gpsimd.dma_start(out=xt_sb[:, 3, :], in_=xt[:, 3, :])
                nc.sync.dma_start(out=wq_sb[:, 0], in_=wq[:, 0])
                nc.sync.dma_start(out=wk_sb[:, 0], in_=wk[:, 0])
                nc.gpsimd.dma_start(out=bq_sb[:], in_=bq[:])
                for c in range(4, ND):
                    nc.sync.dma_start(out=xt_sb[:, c, :], in_=xt[:, c, :])
                nc.sync.dma_start(out=wq_sb[:, 1], in_=wq[:, 1])
                nc.sync.dma_start(out=wk_sb[:, 1], in_=wk[:, 1])
                nc.sync.dma_start(out=wv_sb[:], in_=wv[:])
                nc.sync.dma_start(out=wo_sb[:], in_=wo[:])
                nc.gpsimd.dma_start(out=bv_sb[:], in_=bv_bc[:])
                nc.gpsimd.dma_start(out=mask_sb[:], in_=mask01[:])
                nc.gpsimd.dma_start(out=sela_sb[:], in_=sel[0:1, :])
                nc.gpsimd.dma_start(out=selb_sb[:], in_=sel[1:2, :])
                for p in range(NPAIR):
                    vfix_bc = bass.AP(
                        tensor=vfix.ap().tensor,
                        offset=0,
                        ap=[[64, 128], [0, NS16], [1, 64]],
                    )
                    nc.gpsimd.dma_start(out=va_sb[:, p, :, 64:128], in_=vfix_bc)

                # ---- Q^T / K^T projections (per pair, dk on partitions).
                # c-outer so the PE wants xt chunk c only ~1.7us after
                # chunk c-1: it trails right behind the streaming DMA.
                for p in range(NPAIR):
                    qps = [
                        mmp.tile([128, 512], F32, tag="mm", name=f"qp{sb}")
                        for sb in range(NS)
                    ]
                    kps = [
                        mmp.tile([128, 512], F32, tag="mm", name=f"kp{sb}")
                        for sb in range(NS)
                    ]
                    for c in range(ND):
                        for sb in range(NS):
                            nc.tensor.matmul(
                                qps[sb][:],
                                lhsT=wq_sb[:, p, c, :],
                                rhs=xt_sb[:, c, sb * 512:(sb + 1) * 512],
                                start=(c == 0),
                                stop=(c == ND - 1),
                            )
                        for sb in range(NS):
                            nc.tensor.matmul(
                                kps[sb][:],
                                lhsT=wk_sb[:, p, c, :],
                                rhs=xt_sb[:, c, sb * 512:(sb + 1) * 512],
                                start=(c == 0),
                                stop=(c == ND - 1),
                            )
                    for sb in range(NS):
                        nc.scalar.activation(
                            out=qt_sb[:, p, sb * 512:(sb + 1) * 512],
                            in_=qps[sb][:],
                            func=mybir.ActivationFunctionType.Identity,
                            bias=bq_sb[:, p:p + 1],
                            scale=1.0,
                        )
                        nc.vector.tensor_copy(
                            out=kt_sb[:, p, sb * 512:(sb + 1) * 512],
                            in_=kps[sb][:],
                        )

            # ---- attention + output projection, per query block.
            # The two heads of a pair run TOGETHER chunk-by-chunk: their
            # scores share one 2-bank PSUM tile (even head bank 0, odd
            # bank 1), one rectangular strided ACT instruction
            # exponentiates both with zero wasted columns, one DVE op
            # masks both diagonal triangles, and the pair's softmax
            # normalization shares a single sel-matmul broadcast +
            # reciprocal. Output-projection slots for the previous qb are
            # spread evenly through the score/ctx stream so the PE fills
            # the gaps that the exp pipeline leaves.
            with (
                tc.tile_pool(name="stp", bufs=2, space="PSUM") as stp,
                tc.tile_pool(name="ctxp", bufs=2, space="PSUM") as ctxp,
                tc.tile_pool(name="vpp", bufs=1, space="PSUM") as vpp,
                tc.tile_pool(name="ptp", bufs=6) as ptp,
                tc.tile_pool(name="smp", bufs=3) as smp,
                tc.tile_pool(name="outp", bufs=3) as outp,
            ):
                def emit_vblock(s0):
                    # V projection super-block (two seq row-blocks in one
                    # PSUM bank), in natural [s, dk] layout, 4 heads at
                    # once. Interleaved into the attention stream: qb's
                    # ctx only needs va rows < (qb+1)*4, so block s0 for
                    # qb+1 rides inside qb's score/ctx stream, filling
                    # the PE gaps the exp pipeline leaves. bv is added
                    # during the eviction (tensor_add with a
                    # partition-broadcast constant): exact through the
                    # softmax denominator trick since rows of P sum to
                    # den.
                    vp = vpp.tile([128, 2, 256], F32, tag="vp", name="vp")
                    for bi in range(2):
                        s16 = s0 + bi
                        for c in range(ND):
                            nc.tensor.matmul(
                                vp[:, bi, :],
                                lhsT=xt_sb[:, c, s16 * 128:(s16 + 1) * 128],
                                rhs=wv_sb[:, c, :],
                                start=(c == 0),
                                stop=(c == ND - 1),
                            )
                        # V_even -> va cols 0:64, V_odd -> cols 128:192
                        # in one two-segment add per pair
                        for p2 in range(NPAIR):
                            d0 = va_sb[:, p2, s16, 0:64]
                            dst = bass.AP(
                                tensor=d0.tensor, offset=d0.offset,
                                ap=[[d0.ap[0][0], 128], [128, 2], [1, 64]],
                            )
                            s0v = vp[:, bi, p2 * 128:(p2 + 1) * 128]
                            srcv = bass.AP(
                                tensor=s0v.tensor, offset=s0v.offset,
                                ap=[[s0v.ap[0][0], 128], [64, 2], [1, 64]],
                            )
                            nc.vector.tensor_add(
                                out=dst, in0=srcv, in1=bv_sb[:, p2, :, :]
                            )
                def emit_norm_pair(ctx_e, ctx_o, den_e, den_o, p, qb):
                    # two rank-1 broadcast matmuls serve both heads: sel_a
                    # routes den_e to partitions 0:64, sel_b routes den_o
                    # to 64:128, matching the ctx parity layout. The two
                    # den copies live in separate tiles so DVE and ACT
                    # produce them in parallel. custom-DVE ops (and
                    # tile_position=(0,64) matmuls) misbehave on HW when
                    # based at partition 64, so everything stays at base 0.
                    bc_ps = ctxp.tile([128, 512], F32, tag="op", name="bc_ps", bufs=1)
                    nc.tensor.matmul(
                        bc_ps[:],
                        lhsT=sela_sb[:],
                        rhs=den_e[:],
                        start=True,
                        stop=False,
                    )
                    nc.tensor.matmul(
                        bc_ps[:],
                        lhsT=selb_sb[:],
                        rhs=den_o[:],
                        start=False,
                        stop=True,
                    )
                    rcp = smp.tile([128, 512], F32, tag="rcp", name="rcp")
                    nc.vector.reciprocal_approx_fast(out=rcp[:], in_=bc_ps[:])
                    nc.vector.tensor_mul(
                        out=ctxcat_sb[0:64, p, qb * 512:(qb + 1) * 512],
                        in0=ctx_e[0:64, :],
                        in1=rcp[0:64, :],
                    )
                    nc.vector.tensor_mul(
                        out=ctxcat_sb[64:128, p, qb * 512:(qb + 1) * 512],
                        in0=ctx_o[64:128, :],
                        in1=rcp[64:128, :],
                    )

                ot_tiles = {}

                def emit_op_slot(s16, do, alt=False):
                    # one (row-block, output-half) slice of the output
                    # projection: two accumulating matmuls, an eviction,
                    # and (on the second half) the store. alt=True draws
                    # the PSUM bank from the (by then idle) V ring so the
                    # final back-to-back burst double-buffers across the
                    # two single-buffer rings.
                    if do == 0:
                        ot_tiles[s16] = outp.tile(
                            [128, D], F16, tag="ot", name="ot"
                        )
                    ot = ot_tiles[s16]
                    if alt:
                        op = vpp.tile([128, 512], F32, tag="vp", name="opv", bufs=1)
                    else:
                        op = ctxp.tile([128, 512], F32, tag="op", name="op", bufs=1)
                    nc.tensor.matmul(
                        op[:],
                        lhsT=ctxcat_sb[:, 0, s16 * 128:(s16 + 1) * 128],
                        rhs=wo_sb[:, 0, do * 512:(do + 1) * 512],
                        start=True,
                        stop=False,
                    )
                    nc.tensor.matmul(
                        op[:],
                        lhsT=ctxcat_sb[:, 1, s16 * 128:(s16 + 1) * 128],
                        rhs=wo_sb[:, 1, do * 512:(do + 1) * 512],
                        start=False,
                        stop=True,
                    )
                    if do == 0:
                        # both halves evict on DVE: ACT is the pace-setter
                        # of the long score sweeps these slots ride in
                        nc.vector.tensor_copy(out=ot[:, 0:512], in_=op[:])
                    else:
                        nc.vector.tensor_copy(out=ot[:, 512:1024], in_=op[:])
                        oq = nc.sync if s16 % 2 == 0 else nc.scalar
                        oq.dma_start(
                            out=out[s16 * 128:(s16 + 1) * 128, :], in_=ot[:]
                        )

                emit_vblock(0)
                pending = None  # (ctx_e, ctx_o, den_e, den_o, p, qb)
                slots = []  # outproj work carried across query blocks
                for qb in range(NS):
                    nch = (qb + 1) * 4
                    if qb > 0:
                        slots += [(s16, do)
                                  for s16 in range((qb - 1) * 4, qb * 4)
                                  for do in range(2)]
                    # later query blocks have longer ACT-gated score
                    # sweeps and no V filler, so save outproj slots for
                    # them: drain ~6 per middle qb, everything in the last
                    want = len(slots) if qb == NS - 1 else min(len(slots), 6)
                    stride = max(1, (2 * nch) // want) if want else 1
                    drained = 0
                    for p in range(NPAIR):
                        qs_e = qt_sb[0:64, p, qb * 512:(qb + 1) * 512]
                        qs_o = qt_sb[64:128, p, qb * 512:(qb + 1) * 512]
                        ctx_e = ctxp.tile([128, 512], F32, tag="ctx", name="ctx_e")
                        ctx_o = ctxp.tile([128, 512], F32, tag="ctx", name="ctx_o")
                        lagged = []  # (pt, f0, diag, c) awaiting ctx mms

                        def emit_ctx(lag, ctx_e=ctx_e, ctx_o=ctx_o, p=p, nch=nch):
                            pt, f0, diag, c = lag
                            oe = ctx_e[0:65, f0:512] if diag else ctx_e[0:65, :]
                            nc.tensor.matmul(
                                oe,
                                lhsT=va_sb[:, p, c, 0:65],
                                rhs=pt[:, f0:512],
                                start=(c == 0),
                                stop=(c == nch - 1),
                            )
                            oo = ctx_o[:, f0:512] if diag else ctx_o[:]
                            nc.tensor.matmul(
                                oo,
                                lhsT=va_sb[:, p, c, 64:192],
                                rhs=pt[:, 512 + f0:1024],
                                start=(c == 0),
                                stop=(c == nch - 1),
                            )

                        for c in range(nch):
                            st = stp.tile([128, 1024], F32, tag="st", name="st")
                            pt = ptp.tile([128, 1024], BF16, tag="pt", name="pt")
                            diag = c >= qb * 4
                            # columns [0, f0) of a diag block are fully
                            # masked (q < kv everywhere): skip them.
                            f0 = 128 * (c - qb * 4) if diag else 0
                            nc.tensor.matmul(
                                st[:, f0:512],
                                lhsT=kt_sb[0:64, p, c * 128:(c + 1) * 128],
                                rhs=qs_e[:, f0:512],
                                start=True,
                                stop=True,
                            )
                            nc.tensor.matmul(
                                st[:, 512 + f0:1024],
                                lhsT=kt_sb[64:128, p, c * 128:(c + 1) * 128],
                                rhs=qs_o[:, f0:512],
                                start=True,
                                stop=True,
                            )
                            # one exp over both heads' halves: rectangular
                            # strided AP, no garbage columns
                            sv = st[:, f0:512]
                            pv = pt[:, f0:512]
                            nc.scalar.activation(
                                out=bass.AP(
                                    tensor=pv.tensor, offset=pv.offset,
                                    ap=[[pv.ap[0][0], 128], [512, 2],
                                        [1, 512 - f0]],
                                ),
                                in_=bass.AP(
                                    tensor=sv.tensor, offset=sv.offset,
                                    ap=[[sv.ap[0][0], 128], [512, 2],
                                        [1, 512 - f0]],
                                ),
                                func=mybir.ActivationFunctionType.Exp,
                            )
                            if diag:
                                # zero both heads' masked triangles in one
                                # DVE op; masking precedes the ones-column
                                # ctx matmul so the denominator stays exact
                                blk = pt[:, f0:f0 + 128]
                                mdst = bass.AP(
                                    tensor=blk.tensor, offset=blk.offset,
                                    ap=[[blk.ap[0][0], 128], [512, 2],
                                        [1, 128]],
                                )
                                msk = bass.AP(
                                    tensor=mask_sb.tensor,
                                    offset=mask_sb.offset,
                                    ap=[[mask_sb.ap[0][0], 128], [0, 2],
                                        [1, 128]],
                                )
                                nc.vector.tensor_mul(
                                    out=mdst, in0=mdst, in1=msk
                                )
                            # ctx trails four chunks behind its exp so the
                            # PE queue stays deep (hides ACT latency,
                            # semaphore propagation, and LDWEIGHTS)
                            lagged.append((pt, f0, diag, c))
                            while len(lagged) > 4:
                                emit_ctx(lagged.pop(0))
                            nf = 2 if nch <= 4 else 4
                            if c == nf and pending is not None:
                                emit_norm_pair(*pending)
                                pending = None
                            g = p * nch + c
                            if (drained < want and g >= 5
                                    and (g - 5) % stride == 0):
                                emit_op_slot(*slots.pop(0))
                                drained += 1
                            if qb == 0 and p == 0 and c == 1:
                                # second half of qb0's V rides inside its
                                # own score stream (fills the wait for the
                                # first super-block's DVE evictions)
                                emit_vblock(2)
                            if c == nch // 2 and qb < NS - 1:
                                # one V super-block for qb+1 per pair
                                emit_vblock((qb + 1) * 4 + p * 2)
                        for lag in lagged:
                            emit_ctx(lag)

                        # denominator rows -> two independent [1,512]
                        # tiles so the copies run in parallel (even on
                        # DVE, odd on ACT) and nothing serializes the
                        # broadcast matmuls at the next pair boundary
                        den_e = smp.tile([1, 512], BF16, tag="den", name="den_e")
                        den_o = smp.tile([1, 512], BF16, tag="deno", name="den_o")
                        nc.vector.tensor_copy(
                            out=den_e[:], in_=ctx_e[64:65, :]
                        )
                        nc.scalar.copy(
                            out=den_o[:], in_=ctx_o[0:1, :]
                        )
                        pending = (ctx_e, ctx_o, den_e, den_o, p, qb)

                emit_norm_pair(*pending)
                pending = None
                k = 0
                for s16, do in slots:
                    emit_op_slot(s16, do, alt=(k % 2 == 1))
                    k += 1
                for s16 in range((NS - 1) * 4, NS * 4):
                    emit_op_slot(s16, 0, alt=(k % 2 == 1))
                    k += 1
                    emit_op_slot(s16, 1, alt=(k % 2 == 1))
                    k += 1
    if not nc.is_finalized():
        nc.finalize()
    return nc


def _prep_inputs(embeddings, Wq, bq, Wk, bk, Wv, bv, Wo, bo):
    embeddings = np.asarray(embeddings, np.float32)
    Wq, bq = np.asarray(Wq, np.float32), np.asarray(bq, np.float32)
    Wk = np.asarray(Wk, np.float32)
    Wv, bv = np.asarray(Wv, np.float32), np.asarray(bv, np.float32)
    Wo = np.asarray(Wo, np.float32)

    import ml_dtypes
    bf16_t = ml_dtypes.bfloat16
    # mask01[p, j] = 1 where col j (query) >= partition p (key) in the
    # 128x128 diagonal block, else 0
    mask01 = np.triu(np.ones((128, 128), np.float32)).astype(bf16_t)
    vfix = np.zeros((128, 64), np.float32)
    # two ones columns: va col 64 puts the denominator on partition 64 for
    # even heads / partition 0 for odd heads; va col 65 duplicates it on
    # partition 1 so the odd-head den copy needs no partition shift
    vfix[:, 0] = 1.0
    vfix[:, 1] = 1.0
    vfix = vfix.astype(bf16_t)
    sel = np.zeros((2, 128), np.float32)
    sel[0, 0:64] = 1.0
    sel[1, 64:128] = 1.0
    sel = sel.astype(bf16_t)

    in_maps = []
    for c in range(NCORES):
        b, g = c // 4, c % 4
        hs = HPC * g
        # [128, ND, S]: partition = d % 128, chunk = d // 128
        xt = np.ascontiguousarray(
            embeddings[b].T.reshape(ND, 128, S).transpose(1, 0, 2)
        ).astype(bf16_t)
        # 1/sqrt(dk) folded into Wq/bq (exact power of two)
        wq2 = np.stack(
            [np.concatenate([Wq[hs + 2 * p], Wq[hs + 2 * p + 1]], axis=1)
             for p in range(NPAIR)]
        ) * 0.125
        wk2 = np.stack(
            [np.concatenate([Wk[hs + 2 * p], Wk[hs + 2 * p + 1]], axis=1)
             for p in range(NPAIR)]
        )
        # [NPAIR, D, 128] -> [128, NPAIR, ND, 128] single-DMA layout
        wq2 = wq2.reshape(NPAIR, ND, 128, 128).transpose(2, 0, 1, 3)
        wk2 = wk2.reshape(NPAIR, ND, 128, 128).transpose(2, 0, 1, 3)
        wv4 = np.concatenate([Wv[hs + h] for h in range(HPC)], axis=1)
        wv4 = wv4.reshape(ND, 128, 256).transpose(1, 0, 2)
        wo4 = Wo[hs * DK:(hs + HPC) * DK, :].reshape(2, 128, D).transpose(1, 0, 2)
        bq2 = np.stack(
            [np.concatenate([bq[hs + 2 * p], bq[hs + 2 * p + 1]]) / 8.0
             for p in range(NPAIR)], axis=1
        )
        bvb = np.zeros((128, NPAIR, 2, 64), np.float32)
        for p in range(NPAIR):
            bvb[:, p, 0, :] = bv[hs + 2 * p][None, :]
            bvb[:, p, 1, :] = bv[hs + 2 * p + 1][None, :]
        in_maps.append({
            "xt": xt,
            "wq": np.ascontiguousarray(wq2).astype(bf16_t),
            "wk": np.ascontiguousarray(wk2).astype(bf16_t),
            "wv": np.ascontiguousarray(wv4).astype(bf16_t),
            "wo": np.ascontiguousarray(wo4).astype(bf16_t),
            "bq": np.ascontiguousarray(bq2),
            "bv_bc": bvb,
            "mask01": mask01,
            "sel": sel,
            "vfix": vfix,
        })
    return in_maps


def kernel(embeddings, Wq, bq, Wk, bk, Wv, bv, Wo, bo, _trace=False, _trace_kw=None):
    if "nc" not in _CACHE:
        _CACHE["nc"] = _build_bass()
    nc = _CACHE["nc"]
    in_maps = _prep_inputs(embeddings, Wq, bq, Wk, bk, Wv, bv, Wo, bo)
    kw = dict(_trace_kw or {})
    res = run_bass_kernel_spmd(
        nc, in_maps, core_ids=list(range(NCORES)), trace=_trace, **kw
    )
    _CACHE["last_result"] = res
    bo32 = np.asarray(bo, np.float32)
    out = np.empty((B, S, D), np.float32)
    for b in range(B):
        acc = np.asarray(res.results[4 * b]["out"], np.float32).copy()
        for g in range(1, 4):
            acc += np.asarray(res.results[4 * b + g]["out"], np.float32)
        out[b] = acc + bo32
    return out
